# revision 22
# baseline (speedup 1.0000x reference)
"""MultiHeadGraphAttention TRN2 kernel, v2.

Data-parallel over (batch, query-half): core c handles batch c//2, query rows
(c%2)*1024 .. +1024.  All matmuls bf16 (fp32 PSUM); softmax + LayerNorm fp32.

v2 changes vs baseline (337us):
 - ScalarE is the wall (~130us of exp).  Everything else is arranged to hide
   under it: PSUM->SBUF projection copies moved to DVE, LayerNorm rstd uses
   ln+exp (both in the natural_log_exp_and_others table set -> no table
   thrash; Sqrt previously forced 10 table reloads mid-kernel and stalled the
   exp stream).
 - Score matmuls of a head PAIR run concurrently on disjoint PE row halves
   (K=64 each; tile_position auto-derived from base partitions 0/64).
 - Attention inner loop is software-pipelined: AV matmuls of group g-1 are
   emitted after the score matmuls of group g, so the in-order PE queue never
   blocks the next score tile (and the exp stream) behind a mask-waiting AV.
 - Input DMAs are split per consumption chunk and emitted in consumption
   order; projections start as soon as their inputs land (~4us) instead of
   after all input DMA (~38us).  Remaining projections are threaded into the
   attention stream as PE filler so the PE never idles > ~1us (HAM stays at
   K=8/8).
 - softmax denominator from an appended ones-column on V (row 64 of the AV
   output); reciprocal on DVE, partition-broadcast + normalize mul on GPSIMD.
"""

import os
import sys

import numpy as np

try:
    import concourse  # noqa: F401
except ImportError:  # harness runs from a bare dir; the repo is a fixed path
    sys.path.insert(0, "/opt/trn_rl_repo")

import ml_dtypes

B, N, M, D, H, HD = 4, 2048, 2048, 512, 8, 64
NS = 1024          # query rows per core
NCORES = 8
LN_EPS = 1e-5
BF16 = ml_dtypes.bfloat16

_CACHE = {}

# fallback knobs (read once at build)
# NOTE: reciprocal_approx_fast passes CoreSim but returns garbage on HW.
# NOTE: GPSIMD cannot access PSUM (BIR verifier) -> PSUM-reading ops on DVE.
K_XT = int(os.environ.get("K_XT", "0"))   # x_t add on gpsimd vs vector


def _build():
    import concourse.bass as bass  # noqa: F401
    import concourse.tile as tile
    from concourse import bacc, mybir
    from concourse.masks import make_identity

    f32 = mybir.dt.float32
    bf16 = mybir.dt.bfloat16
    Exp = mybir.ActivationFunctionType.Exp
    Sqrt = mybir.ActivationFunctionType.Sqrt
    sub = mybir.AluOpType.subtract
    mult = mybir.AluOpType.mult

    nc = bacc.Bacc(None, target_bir_lowering=False, debug=False)

    xqT_d = nc.dram_tensor("xqT", [D, NS], bf16, kind="ExternalInput")
    xkT_d = nc.dram_tensor("xkT", [D, M], bf16, kind="ExternalInput")
    xvT_d = nc.dram_tensor("xvT", [D, M], bf16, kind="ExternalInput")
    maskP_d = nc.dram_tensor("maskP", [2 * 8 * 128, 1024], bf16, kind="ExternalInput")
    qres_d = nc.dram_tensor("qres", [NS, D], f32, kind="ExternalInput")
    wqT_d = nc.dram_tensor("wqT", [D, D], bf16, kind="ExternalInput")
    wkT_d = nc.dram_tensor("wkT", [D, D], bf16, kind="ExternalInput")
    wvT_d = nc.dram_tensor("wvT", [D, D], bf16, kind="ExternalInput")
    woT_d = nc.dram_tensor("woT", [D, D], bf16, kind="ExternalInput")
    gamma_d = nc.dram_tensor("gamma", [1, D], f32, kind="ExternalInput")
    beta_d = nc.dram_tensor("beta", [1, D], f32, kind="ExternalInput")
    out_d = nc.dram_tensor("out", [NS, D], f32, kind="ExternalOutput")

    KC = D // 128      # 4 contraction chunks of 128
    NCH = NS // 512    # 2 query-column chunks
    MT = M // 128      # 16 key-position tiles
    MCH = M // 512     # 4 key chunks of 512
    MG = MT // 2       # 8 score groups (2 key tiles per group)
    HW = HD + 1        # per-head V slot width (64 V cols + ones col)

    with tile.TileContext(nc) as tc:
        with (
            tc.tile_pool(name="big", bufs=1) as big,
            tc.tile_pool(name="wpool", bufs=1) as wpool,
            tc.tile_pool(name="ppool", bufs=4) as ppool,
            tc.tile_pool(name="xpool", bufs=5) as xpool,
            tc.tile_pool(name="mvpool", bufs=6) as mvpool,
            tc.tile_pool(name="ypool", bufs=3) as ypool,
            tc.tile_pool(name="rpool", bufs=2) as rpool,
            tc.tile_pool(name="small", bufs=6) as small,
            tc.tile_pool(name="ps_mm", bufs=2, space="PSUM") as ps_mm,
            tc.tile_pool(name="ps_sA", bufs=1, space="PSUM") as ps_sA,
            tc.tile_pool(name="ps_sB", bufs=1, space="PSUM") as ps_sB,
            tc.tile_pool(name="ps_o", bufs=1, space="PSUM") as ps_o,
        ):
            # ---- resident SBUF tensors -----------------------------------
            xqT = big.tile([128, KC, NS], bf16, tag="xqT")
            xkT = big.tile([128, KC, M], bf16, tag="xkT")
            xvT = big.tile([128, KC, M], bf16, tag="xvT")
            maskS = big.tile([128, NCH, MG, 1024], bf16, tag="maskS")
            qT = big.tile([128, KC, NS], bf16, tag="qT")
            kT = big.tile([128, KC, M], bf16, tag="kT")
            vS = big.tile([128, MT, H * HW], bf16, tag="vS")
            oT = big.tile([128, KC, NS], bf16, tag="oT")
            wq = wpool.tile([128, KC, D], bf16, tag="wq")
            wk = wpool.tile([128, KC, D], bf16, tag="wk")
            wv = wpool.tile([128, KC, D], bf16, tag="wv")
            wo = wpool.tile([128, KC, D], bf16, tag="wo")
            gamma_b = wpool.tile([128, D], f32, tag="gamma_b")
            beta_b = wpool.tile([128, D], f32, tag="beta_b")
            gamma_1 = wpool.tile([1, D], f32, tag="gamma_1")
            beta_1 = wpool.tile([1, D], f32, tag="beta_1")
            eps_t = wpool.tile([128, 1], f32, tag="eps")
            ident = wpool.tile([128, 128], f32, tag="ident")
            make_identity(nc, ident)

            # ---- setup (no DMA dependencies; engines idle early) ---------
            nc.vector.memset(eps_t, LN_EPS)
            # ones column per head in the augmented V (softmax denominator
            # lands as row 64 of the AV matmul output)
            nc.vector.memset(
                vS[:].rearrange("p j (h x) -> p j h x", x=HW)[:, :, :, HD : HD + 1],
                1.0,
            )

            # ---- input DMAs, split per consumption chunk, priority order -
            xq_r = xqT_d[:].rearrange("(c p) n -> p c n", p=128)
            xk_r = xkT_d[:].rearrange("(c p) n -> p c n", p=128)
            xv_r = xvT_d[:].rearrange("(c p) n -> p c n", p=128)
            mk_r = maskP_d[:].rearrange("(c g p) n -> p c g n", c=NCH, g=MG)

            nc.sync.dma_start(out=wq, in_=wqT_d[:].rearrange("(c p) o -> p c o", p=128))
            for ncc in range(NCH):
                sl = slice(ncc * 512, (ncc + 1) * 512)
                nc.sync.dma_start(out=xqT[:, :, sl], in_=xq_r[:, :, sl])
            nc.sync.dma_start(out=wk, in_=wkT_d[:].rearrange("(c p) o -> p c o", p=128))
            for mc in range(MCH):
                sl = slice(mc * 512, (mc + 1) * 512)
                nc.sync.dma_start(out=xkT[:, :, sl], in_=xk_r[:, :, sl])
            nc.sync.dma_start(out=maskS[:, 0, 0, :], in_=mk_r[:, 0, 0, :])
            nc.sync.dma_start(out=maskS[:, 0, 1, :], in_=mk_r[:, 0, 1, :])
            nc.sync.dma_start(out=wv, in_=wvT_d[:].rearrange("(c p) o -> p c o", p=128))
            for jc in range(4):
                sl = slice(jc * 256, (jc + 1) * 256)
                nc.sync.dma_start(out=xvT[:, :, sl], in_=xv_r[:, :, sl])
            nc.sync.dma_start(out=maskS[:, 0, 2, :], in_=mk_r[:, 0, 2, :])
            nc.sync.dma_start(out=maskS[:, 0, 3, :], in_=mk_r[:, 0, 3, :])
            for jc in range(4, 8):
                sl = slice(jc * 256, (jc + 1) * 256)
                nc.sync.dma_start(out=xvT[:, :, sl], in_=xv_r[:, :, sl])
            for g in range(4, MG):
                nc.sync.dma_start(out=maskS[:, 0, g, :], in_=mk_r[:, 0, g, :])
            nc.sync.dma_start(out=wo, in_=woT_d[:].rearrange("(c p) o -> p c o", p=128))
            for g in range(MG):
                nc.sync.dma_start(out=maskS[:, 1, g, :], in_=mk_r[:, 1, g, :])
            nc.sync.dma_start(out=gamma_1, in_=gamma_d[:])
            nc.sync.dma_start(out=beta_1, in_=beta_d[:])
            nc.gpsimd.partition_broadcast(gamma_b, gamma_1, channels=128)
            nc.gpsimd.partition_broadcast(beta_b, beta_1, channels=128)

            # ---- projection emitters (PSUM->SBUF copies on DVE) ----------
            def q_proj(t, ncc):
                ps = ps_mm.tile([128, 512], f32, tag="mm")
                sl = slice(ncc * 512, (ncc + 1) * 512)
                for kc in range(KC):
                    nc.tensor.matmul(
                        ps,
                        lhsT=wq[:, kc, t * 128 : (t + 1) * 128],
                        rhs=xqT[:, kc, sl],
                        start=(kc == 0),
                        stop=(kc == KC - 1),
                    )
                nc.vector.tensor_copy(out=qT[:, t, sl], in_=ps)

            def k_proj(t, mc):
                ps = ps_mm.tile([128, 512], f32, tag="mm")
                sl = slice(mc * 512, (mc + 1) * 512)
                for kc in range(KC):
                    nc.tensor.matmul(
                        ps,
                        lhsT=wk[:, kc, t * 128 : (t + 1) * 128],
                        rhs=xkT[:, kc, sl],
                        start=(kc == 0),
                        stop=(kc == KC - 1),
                    )
                nc.vector.tensor_copy(out=kT[:, t, sl], in_=ps)

            def v_proj(j):
                # V[m, o] straight, scattered into per-head 65-wide slots
                ps = ps_mm.tile([128, 512], f32, tag="mm")
                for kc in range(KC):
                    nc.tensor.matmul(
                        ps,
                        lhsT=xvT[:, kc, j * 128 : (j + 1) * 128],
                        rhs=wv[:, kc, :],
                        start=(kc == 0),
                        stop=(kc == KC - 1),
                    )
                nc.vector.tensor_copy(
                    out=vS[:, j, :].rearrange("p (h x) -> p h x", x=HW)[:, :, 0:HD],
                    in_=ps[:].rearrange("p (h x) -> p h x", x=HD),
                )

            # ---- attention: head pair 2t/2t+1, software-pipelined --------
            def normalize(po_t, h, t, nsl):
                # copy U^T and the denominator row out of PSUM right away
                # (frees the AV bank for the next pair); reciprocal via the
                # PE-transpose dance — DVE reciprocal is ~6 cycles/elem along
                # the FREE dim, so [128,4] (0.2us) beats [.,512] (3us)
                po2 = (h % 2) * 64
                uS = rpool.tile([64, 512], f32, tag="uS")
                nc.vector.tensor_copy(out=uS, in_=po_t[0:HD, :])
                dS = rpool.tile([1, 512], f32, tag="dS")
                nc.vector.tensor_copy(out=dS, in_=po_t[HD : HD + 1, :])
                scr = ps_mm.tile([128, 512], f32, tag="mm")
                dT = scr[:, 0:4]
                rrow = scr[0:1, 0:512]
                for c in range(KC):
                    nc.tensor.transpose(
                        dT[:, c : c + 1], dS[:, c * 128 : (c + 1) * 128],
                        ident[0:1, 0:1],
                    )
                rT = small.tile([128, 4], f32, tag="rT")
                nc.vector.reciprocal(rT, dT)
                for c in range(KC):
                    nc.tensor.transpose(
                        rrow[:, c * 128 : (c + 1) * 128], rT[:, c : c + 1], ident
                    )
                recip_s = rpool.tile([1, 512], f32, tag="recip")
                nc.vector.tensor_copy(out=recip_s, in_=rrow)
                rb = rpool.tile([64, 512], f32, tag="rb")
                nc.gpsimd.partition_broadcast(rb, recip_s, channels=64)
                nc.gpsimd.tensor_mul(oT[po2 : po2 + 64, t, nsl], uS, rb)

            # one continuous stream over all (pair, chunk, group) units; the
            # AV matmuls run one unit behind the score/exp/mask front so the
            # in-order PE queue never blocks the exp stream, and the pipeline
            # never drains at pair boundaries
            pend = {}   # live pair state: (t, ncc) -> (poE, poO, nsl)
            pts = {}    # unit -> (ptA, ptB)

            def emit_av(unit):
                t, ncc, g = unit
                poE, poO, _ = pend[(t, ncc)]
                slotE = slice((2 * t) * HW, (2 * t + 1) * HW)
                slotO = slice((2 * t + 1) * HW, (2 * t + 2) * HW)
                ptA, ptB = pts.pop(unit)
                for u in range(2):
                    j = 2 * g + u
                    usl = slice(u * 512, (u + 1) * 512)
                    nc.tensor.matmul(
                        poE, lhsT=vS[:, j, slotE], rhs=ptA[:, usl],
                        start=(j == 0), stop=(j == MT - 1),
                    )
                    nc.tensor.matmul(
                        poO, lhsT=vS[:, j, slotO], rhs=ptB[:, usl],
                        start=(j == 0), stop=(j == MT - 1),
                    )
                if g == MG - 1:
                    poE, poO, nsl = pend.pop((t, ncc))
                    normalize(poE, 2 * t, t, nsl)
                    normalize(poO, 2 * t + 1, t, nsl)

            def attend_all(pair_order, fillmap):
                units = [(t, ncc, g) for (t, ncc) in pair_order for g in range(MG)]
                prev = None
                for unit in units:
                    t, ncc, g = unit
                    nsl = slice(ncc * 512, (ncc + 1) * 512)
                    if g == 0:
                        poE_new = ps_o.tile([HW, 512], f32, tag="poE")
                        poO_new = ps_o.tile([HW, 512], f32, tag="poO")
                        pend[(t, ncc)] = (poE_new, poO_new, nsl)
                    poE, poO, _ = pend[(t, ncc)]
                    psA = ps_sA.tile([128, 1024], f32, tag="sA")
                    psB = ps_sB.tile([128, 1024], f32, tag="sB")
                    for u in range(2):
                        j = 2 * g + u
                        usl = slice(u * 512, (u + 1) * 512)
                        # two heads on disjoint PE row halves -> concurrent
                        nc.tensor.matmul(
                            psA[:, usl],
                            lhsT=kT[0:64, t, j * 128 : (j + 1) * 128],
                            rhs=qT[0:64, t, nsl],
                            start=True, stop=True,
                        )
                        nc.tensor.matmul(
                            psB[:, usl],
                            lhsT=kT[64:128, t, j * 128 : (j + 1) * 128],
                            rhs=qT[64:128, t, nsl],
                            start=True, stop=True,
                        )
                    ptA = ppool.tile([128, 1024], bf16, tag="pt")
                    nc.scalar.activation(ptA, psA, Exp)
                    ptB = ppool.tile([128, 1024], bf16, tag="pt")
                    nc.scalar.activation(ptB, psB, Exp)
                    nc.vector.tensor_mul(ptA, ptA, maskS[:, ncc, g, :])
                    nc.vector.tensor_mul(ptB, ptB, maskS[:, ncc, g, :])
                    pts[unit] = (ptA, ptB)
                    for f in fillmap.get((t, ncc), {}).get(g, ()):
                        f()
                    if prev is not None:
                        emit_av(prev)
                    prev = unit
                emit_av(prev)

            # ---- output projection + residual + LayerNorm ----------------
            qres_r = qres_d[:].rearrange("(t p) d -> p t d", p=128)
            out_r = out_d[:].rearrange("(t p) d -> p t d", p=128)
            ot_state = {}

            def out_front(nt):
                ps = ps_mm.tile([128, 512], f32, tag="mm")
                for a in range(KC):
                    nc.tensor.matmul(
                        ps,
                        lhsT=oT[:, a, nt * 128 : (nt + 1) * 128],
                        rhs=wo[:, a, :],
                        start=(a == 0),
                        stop=(a == KC - 1),
                    )
                qres_t = ypool.tile([128, D], f32, tag="qres")
                nc.sync.dma_start(out=qres_t, in_=qres_r[:, nt, :])
                x_t = xpool.tile([128, D], f32, tag="x")
                if K_XT:
                    nc.gpsimd.tensor_add(x_t, ps, qres_t)
                else:
                    nc.vector.tensor_add(x_t, ps, qres_t)
                stats = small.tile([128, 6], f32, tag="stats")
                nc.vector.bn_stats(out=stats, in_=x_t)
                mv = mvpool.tile([128, 2], f32, tag="mv")
                nc.vector.bn_aggr(out=mv, in_=stats)
                ot_state[nt] = (x_t, mv)

            rstd_store = {}

            def rstd_batch(nts):
                # one Sqrt activation for a wave of tiles -> 2 ACT table
                # switches per wave instead of 2 per tile
                vcol = small.tile([128, 4], f32, tag="vcol")
                for i, nt in enumerate(nts):
                    nc.vector.tensor_copy(out=vcol[:, i : i + 1],
                                          in_=ot_state[nt][1][:, 1:2])
                sd = small.tile([128, 4], f32, tag="sd")
                nc.scalar.activation(sd, vcol, Sqrt, bias=eps_t)
                rs = mvpool.tile([128, 4], f32, tag="rs")
                nc.vector.reciprocal(rs, sd)
                for i, nt in enumerate(nts):
                    rstd_store[nt] = (rs, i)

            def out_back(nt, tail=False):
                x_t, mv = ot_state.pop(nt)
                rs, i = rstd_store.pop(nt)
                xn = ypool.tile([128, D], f32, tag="xn")
                nc.vector.tensor_scalar(
                    out=xn, in0=x_t, scalar1=mv[:, 0:1], scalar2=rs[:, i : i + 1],
                    op0=sub, op1=mult,
                )
                y_t = ypool.tile([128, D], f32, tag="y")
                if tail:  # DVE is free at the tail; gpsimd mul is slower
                    nc.vector.tensor_mul(y_t, xn, gamma_b)
                    nc.vector.tensor_add(y_t, y_t, beta_b)
                else:
                    nc.gpsimd.tensor_mul(y_t, xn, gamma_b)
                    nc.gpsimd.tensor_add(y_t, y_t, beta_b)
                nc.sync.dma_start(out=out_r[:, nt, :], in_=y_t)

            # ---- emission schedule ---------------------------------------
            # ramp: just enough projection work for pair 0 + first AV tiles
            q_proj(0, 0)
            q_proj(0, 1)
            for mc in range(MCH):
                k_proj(0, mc)
            v_proj(0)
            v_proj(1)

            def C(f, *a):
                return lambda: f(*a)

            # pair-0 fillers: V tiles JIT (AV of group g needs v(2g,2g+1);
            # slot g supplies v(2g+2,2g+3)); pair-p prereqs (qT/kT complete)
            # must be emitted before pair p starts
            f00 = {
                0: (C(v_proj, 2), C(v_proj, 3)),
                1: (C(v_proj, 4), C(v_proj, 5)),
                2: (C(v_proj, 6), C(v_proj, 7)),
                3: (C(v_proj, 8), C(v_proj, 9)),
                4: (C(v_proj, 10), C(v_proj, 11)),
                5: (C(v_proj, 12), C(v_proj, 13)),
                6: (C(v_proj, 14), C(v_proj, 15), C(q_proj, 1, 0)),
                7: (C(q_proj, 1, 1), C(k_proj, 1, 0)),
            }
            # k(t,mc) feeds score groups 2mc..2mc+1 of pair t: later chunks
            # can trail into pair t itself as long as they stay 2 groups ahead
            f10 = {
                0: (C(k_proj, 1, 1),),
                1: (C(k_proj, 1, 2), C(k_proj, 1, 3)),
                3: (C(q_proj, 2, 0),),
                4: (C(q_proj, 2, 1),),
                5: (C(k_proj, 2, 0),),
                6: (C(k_proj, 2, 1),),
                7: (C(k_proj, 2, 2), C(k_proj, 2, 3)),
            }
            f20 = {
                0: (C(q_proj, 3, 0),),
                1: (C(q_proj, 3, 1),),
                4: (C(k_proj, 3, 0),),
                5: (C(k_proj, 3, 1),),
                6: (C(k_proj, 3, 2), C(k_proj, 3, 3)),
            }
            f01 = {2: (C(out_front, 0),), 4: (C(out_front, 1),),
                   6: (C(out_front, 2),)}
            f11 = {0: (C(out_front, 3),), 2: (C(rstd_batch, (0, 1, 2, 3)),),
                   4: (C(out_back, 0), C(out_back, 1)),
                   6: (C(out_back, 2), C(out_back, 3))}

            pair_order = [(0, 0), (1, 0), (2, 0), (3, 0),
                          (0, 1), (1, 1), (2, 1), (3, 1)]
            fillmap = {(0, 0): f00, (1, 0): f10, (2, 0): f20,
                       (0, 1): f01, (1, 1): f11}
            attend_all(pair_order, fillmap)
            out_front(4)
            out_front(5)
            out_front(6)
            out_front(7)
            rstd_batch((4, 5, 6, 7))
            out_back(4, tail=True)
            out_back(5, tail=True)
            out_back(6, tail=True)
            out_back(7, tail=True)

    nc.compile()
    return nc


def kernel(**inputs):
    from concourse.bass_utils import run_bass_kernel_spmd

    if "nc" not in _CACHE:
        _CACHE["nc"] = _build()
    nc = _CACHE["nc"]

    query = np.asarray(inputs["query"], dtype=np.float32)
    key = np.asarray(inputs["key"], dtype=np.float32)
    value = np.asarray(inputs["value"], dtype=np.float32)
    mask = np.asarray(inputs["mask"])
    WQ = np.asarray(inputs["WQ"], dtype=np.float32)
    WK = np.asarray(inputs["WK"], dtype=np.float32)
    WV = np.asarray(inputs["WV"], dtype=np.float32)
    WO = np.asarray(inputs["WO"], dtype=np.float32)
    bO = np.asarray(inputs["bO"], dtype=np.float32)
    gamma = np.asarray(inputs["gamma"], dtype=np.float32)
    beta = np.asarray(inputs["beta"], dtype=np.float32)

    scale = np.float32(1.0 / np.sqrt(HD))
    wqT = np.ascontiguousarray(WQ.T * scale).astype(BF16)
    wkT = np.ascontiguousarray(WK.T).astype(BF16)
    wvT = np.ascontiguousarray(WV.T).astype(BF16)
    woT = np.ascontiguousarray(WO.T).astype(BF16)
    gamma_in = gamma.reshape(1, D)
    beta_in = beta.reshape(1, D)
    mask_bin = (mask != 0)

    in_maps = []
    for c in range(NCORES):
        b, n0 = c // 2, (c % 2) * NS
        # mask, transposed and prepacked per (n-chunk, score-group):
        # maskP[ncc, g, p, u*512+nn] = maskT[g*256+u*128+p, ncc*512+nn]
        mT = np.ascontiguousarray(mask_bin[b, n0 : n0 + NS, :].T)  # [M, NS]
        mP = (
            mT.reshape(8, 2, 128, 2, 512)
            .transpose(3, 0, 2, 1, 4)
            .reshape(2 * 8 * 128, 1024)
        )
        in_maps.append({
            "xqT": np.ascontiguousarray(query[b, n0 : n0 + NS, :].T).astype(BF16),
            "xkT": np.ascontiguousarray(key[b].T).astype(BF16),
            "xvT": np.ascontiguousarray(value[b].T).astype(BF16),
            "maskP": np.ascontiguousarray(mP).astype(BF16),
            "qres": np.ascontiguousarray(query[b, n0 : n0 + NS, :] + bO[None, :]),
            "wqT": wqT, "wkT": wkT, "wvT": wvT, "woT": woT,
            "gamma": gamma_in, "beta": beta_in,
        })

    trace = bool(int(os.environ.get("BASS_KERNEL_TRACE", "0")))
    res = run_bass_kernel_spmd(nc, in_maps, core_ids=list(range(NCORES)), trace=trace)
    _CACHE["last_results"] = res

    out = np.empty((B, N, D), dtype=np.float32)
    for c in range(NCORES):
        b, n0 = c // 2, (c % 2) * NS
        out[b, n0 : n0 + NS, :] = res.results[c]["out"]
    return out


# revision 25
# speedup vs baseline: 1.0222x; 1.0222x over previous
"""MultiHeadGraphAttention TRN2 kernel, v2.

Data-parallel over (batch, query-half): core c handles batch c//2, query rows
(c%2)*1024 .. +1024.  All matmuls bf16 (fp32 PSUM); softmax + LayerNorm fp32.

v2 changes vs baseline (337us):
 - ScalarE is the wall (~130us of exp).  Everything else is arranged to hide
   under it: PSUM->SBUF projection copies moved to DVE, LayerNorm rstd uses
   ln+exp (both in the natural_log_exp_and_others table set -> no table
   thrash; Sqrt previously forced 10 table reloads mid-kernel and stalled the
   exp stream).
 - Score matmuls of a head PAIR run concurrently on disjoint PE row halves
   (K=64 each; tile_position auto-derived from base partitions 0/64).
 - Attention inner loop is software-pipelined: AV matmuls of group g-1 are
   emitted after the score matmuls of group g, so the in-order PE queue never
   blocks the next score tile (and the exp stream) behind a mask-waiting AV.
 - Input DMAs are split per consumption chunk and emitted in consumption
   order; projections start as soon as their inputs land (~4us) instead of
   after all input DMA (~38us).  Remaining projections are threaded into the
   attention stream as PE filler so the PE never idles > ~1us (HAM stays at
   K=8/8).
 - softmax denominator from an appended ones-column on V (row 64 of the AV
   output); reciprocal on DVE, partition-broadcast + normalize mul on GPSIMD.
"""

import os
import sys

import numpy as np

try:
    import concourse  # noqa: F401
except ImportError:  # harness runs from a bare dir; the repo is a fixed path
    sys.path.insert(0, "/opt/trn_rl_repo")

import ml_dtypes

B, N, M, D, H, HD = 4, 2048, 2048, 512, 8, 64
NS = 1024          # query rows per core
NCORES = 8
LN_EPS = 1e-5
BF16 = ml_dtypes.bfloat16

_CACHE = {}

# fallback knobs (read once at build)
# NOTE: reciprocal_approx_fast passes CoreSim but returns garbage on HW.
# NOTE: GPSIMD cannot access PSUM (BIR verifier) -> PSUM-reading ops on DVE.
K_XT = int(os.environ.get("K_XT", "0"))   # x_t add on gpsimd vs vector


def _build():
    import concourse.bass as bass  # noqa: F401
    import concourse.tile as tile
    from concourse import bacc, mybir
    from concourse.masks import make_identity

    f32 = mybir.dt.float32
    bf16 = mybir.dt.bfloat16
    Exp = mybir.ActivationFunctionType.Exp
    Sqrt = mybir.ActivationFunctionType.Sqrt
    sub = mybir.AluOpType.subtract
    mult = mybir.AluOpType.mult

    nc = bacc.Bacc(None, target_bir_lowering=False, debug=False)

    xqT_d = nc.dram_tensor("xqT", [D, NS], bf16, kind="ExternalInput")
    xkT_d = nc.dram_tensor("xkT", [D, M], bf16, kind="ExternalInput")
    xvT_d = nc.dram_tensor("xvT", [D, M], bf16, kind="ExternalInput")
    maskP_d = nc.dram_tensor("maskP", [2 * 8 * 128, 1024], bf16, kind="ExternalInput")
    qres_d = nc.dram_tensor("qres", [NS, D], f32, kind="ExternalInput")
    wqT_d = nc.dram_tensor("wqT", [D, D], bf16, kind="ExternalInput")
    wkT_d = nc.dram_tensor("wkT", [D, D], bf16, kind="ExternalInput")
    wvT_d = nc.dram_tensor("wvT", [D, D], bf16, kind="ExternalInput")
    woT_d = nc.dram_tensor("woT", [D, D], bf16, kind="ExternalInput")
    gamma_d = nc.dram_tensor("gamma", [1, D], f32, kind="ExternalInput")
    beta_d = nc.dram_tensor("beta", [1, D], f32, kind="ExternalInput")
    out_d = nc.dram_tensor("out", [NS, D], f32, kind="ExternalOutput")

    KC = D // 128      # 4 contraction chunks of 128
    NCH = NS // 512    # 2 query-column chunks
    MT = M // 128      # 16 key-position tiles
    MCH = M // 512     # 4 key chunks of 512
    MG = MT // 2       # 8 score groups (2 key tiles per group)
    HW = HD + 1        # per-head V slot width (64 V cols + ones col)

    with tile.TileContext(nc) as tc:
        with (
            tc.tile_pool(name="big", bufs=1) as big,
            tc.tile_pool(name="wpool", bufs=1) as wpool,
            tc.tile_pool(name="ppool", bufs=4) as ppool,
            tc.tile_pool(name="xpool", bufs=5) as xpool,
            tc.tile_pool(name="mvpool", bufs=6) as mvpool,
            tc.tile_pool(name="ypool", bufs=3) as ypool,
            tc.tile_pool(name="rpool", bufs=2) as rpool,
            tc.tile_pool(name="small", bufs=6) as small,
            tc.tile_pool(name="ps_mm", bufs=2, space="PSUM") as ps_mm,
            tc.tile_pool(name="ps_sA", bufs=1, space="PSUM") as ps_sA,
            tc.tile_pool(name="ps_sB", bufs=1, space="PSUM") as ps_sB,
            tc.tile_pool(name="ps_o", bufs=1, space="PSUM") as ps_o,
        ):
            # ---- resident SBUF tensors -----------------------------------
            xqT = big.tile([128, KC, NS], bf16, tag="xqT")
            xkT = big.tile([128, KC, M], bf16, tag="xkT")
            xvT = big.tile([128, KC, M], bf16, tag="xvT")
            maskS = big.tile([128, NCH, MG, 1024], bf16, tag="maskS")
            qT = big.tile([128, KC, NS], bf16, tag="qT")
            kT = big.tile([128, KC, M], bf16, tag="kT")
            vS = big.tile([128, MT, H * HW], bf16, tag="vS")
            oT = big.tile([128, KC, NS], bf16, tag="oT")
            wq = wpool.tile([128, KC, D], bf16, tag="wq")
            wk = wpool.tile([128, KC, D], bf16, tag="wk")
            wv = wpool.tile([128, KC, D], bf16, tag="wv")
            wo = wpool.tile([128, KC, D], bf16, tag="wo")
            gamma_b = wpool.tile([128, D], f32, tag="gamma_b")
            beta_b = wpool.tile([128, D], f32, tag="beta_b")
            gamma_1 = wpool.tile([1, D], f32, tag="gamma_1")
            beta_1 = wpool.tile([1, D], f32, tag="beta_1")
            eps_t = wpool.tile([128, 1], f32, tag="eps")
            ident = wpool.tile([128, 128], f32, tag="ident")
            make_identity(nc, ident)

            # ---- setup (no DMA dependencies; engines idle early) ---------
            nc.vector.memset(eps_t, LN_EPS)
            # ones column per head in the augmented V (softmax denominator
            # lands as row 64 of the AV matmul output)
            nc.vector.memset(
                vS[:].rearrange("p j (h x) -> p j h x", x=HW)[:, :, :, HD : HD + 1],
                1.0,
            )

            # ---- input DMAs, split per consumption chunk, priority order -
            xq_r = xqT_d[:].rearrange("(c p) n -> p c n", p=128)
            xk_r = xkT_d[:].rearrange("(c p) n -> p c n", p=128)
            xv_r = xvT_d[:].rearrange("(c p) n -> p c n", p=128)
            mk_r = maskP_d[:].rearrange("(c g p) n -> p c g n", c=NCH, g=MG)

            nc.sync.dma_start(out=wq, in_=wqT_d[:].rearrange("(c p) o -> p c o", p=128))
            for ncc in range(NCH):
                sl = slice(ncc * 512, (ncc + 1) * 512)
                nc.sync.dma_start(out=xqT[:, :, sl], in_=xq_r[:, :, sl])
            nc.sync.dma_start(out=wk, in_=wkT_d[:].rearrange("(c p) o -> p c o", p=128))
            for mc in range(MCH):
                sl = slice(mc * 512, (mc + 1) * 512)
                nc.sync.dma_start(out=xkT[:, :, sl], in_=xk_r[:, :, sl])
            nc.sync.dma_start(out=maskS[:, 0, 0, :], in_=mk_r[:, 0, 0, :])
            nc.sync.dma_start(out=maskS[:, 0, 1, :], in_=mk_r[:, 0, 1, :])
            nc.sync.dma_start(out=wv, in_=wvT_d[:].rearrange("(c p) o -> p c o", p=128))
            for jc in range(4):
                sl = slice(jc * 256, (jc + 1) * 256)
                nc.sync.dma_start(out=xvT[:, :, sl], in_=xv_r[:, :, sl])
            nc.sync.dma_start(out=maskS[:, 0, 2, :], in_=mk_r[:, 0, 2, :])
            nc.sync.dma_start(out=maskS[:, 0, 3, :], in_=mk_r[:, 0, 3, :])
            for jc in range(4, 8):
                sl = slice(jc * 256, (jc + 1) * 256)
                nc.sync.dma_start(out=xvT[:, :, sl], in_=xv_r[:, :, sl])
            for g in range(4, MG):
                nc.sync.dma_start(out=maskS[:, 0, g, :], in_=mk_r[:, 0, g, :])
            nc.sync.dma_start(out=wo, in_=woT_d[:].rearrange("(c p) o -> p c o", p=128))
            for g in range(MG):
                nc.sync.dma_start(out=maskS[:, 1, g, :], in_=mk_r[:, 1, g, :])
            nc.sync.dma_start(out=gamma_1, in_=gamma_d[:])
            nc.sync.dma_start(out=beta_1, in_=beta_d[:])
            nc.gpsimd.partition_broadcast(gamma_b, gamma_1, channels=128)
            nc.gpsimd.partition_broadcast(beta_b, beta_1, channels=128)

            # ---- projection emitters (PSUM->SBUF copies on DVE) ----------
            def q_proj(t, ncc):
                ps = ps_mm.tile([128, 512], f32, tag="mm")
                sl = slice(ncc * 512, (ncc + 1) * 512)
                for kc in range(KC):
                    nc.tensor.matmul(
                        ps,
                        lhsT=wq[:, kc, t * 128 : (t + 1) * 128],
                        rhs=xqT[:, kc, sl],
                        start=(kc == 0),
                        stop=(kc == KC - 1),
                    )
                nc.vector.tensor_copy(out=qT[:, t, sl], in_=ps)

            def k_proj(t, mc):
                ps = ps_mm.tile([128, 512], f32, tag="mm")
                sl = slice(mc * 512, (mc + 1) * 512)
                for kc in range(KC):
                    nc.tensor.matmul(
                        ps,
                        lhsT=wk[:, kc, t * 128 : (t + 1) * 128],
                        rhs=xkT[:, kc, sl],
                        start=(kc == 0),
                        stop=(kc == KC - 1),
                    )
                nc.vector.tensor_copy(out=kT[:, t, sl], in_=ps)

            def v_proj(j):
                # V[m, o] straight, scattered into per-head 65-wide slots
                ps = ps_mm.tile([128, 512], f32, tag="mm")
                for kc in range(KC):
                    nc.tensor.matmul(
                        ps,
                        lhsT=xvT[:, kc, j * 128 : (j + 1) * 128],
                        rhs=wv[:, kc, :],
                        start=(kc == 0),
                        stop=(kc == KC - 1),
                    )
                nc.vector.tensor_copy(
                    out=vS[:, j, :].rearrange("p (h x) -> p h x", x=HW)[:, :, 0:HD],
                    in_=ps[:].rearrange("p (h x) -> p h x", x=HD),
                )

            # ---- attention: head pair 2t/2t+1, software-pipelined --------
            def normalize_flat(po_t, h, t, nsl):
                # latency-optimized variant for the last pairs: fewer queue
                # hops (no PE transposes, no gpsimd mul) — the 3us one-lane
                # reciprocal is fine when nothing else needs DVE throughput
                po2 = (h % 2) * 64
                uS = rpool.tile([64, 512], f32, tag="uS")
                nc.vector.tensor_copy(out=uS, in_=po_t[0:HD, :])
                dS = rpool.tile([1, 512], f32, tag="dS")
                nc.vector.tensor_copy(out=dS, in_=po_t[HD : HD + 1, :])
                recip_s = rpool.tile([1, 512], f32, tag="recip")
                nc.vector.reciprocal(recip_s, dS)
                rb = rpool.tile([64, 512], f32, tag="rb")
                nc.gpsimd.partition_broadcast(rb, recip_s, channels=64)
                nc.vector.tensor_mul(oT[po2 : po2 + 64, t, nsl], uS, rb)

            def normalize(po_t, h, t, nsl):
                # copy U^T and the denominator row out of PSUM right away
                # (frees the AV bank for the next pair); reciprocal via the
                # PE-transpose dance — DVE reciprocal is ~6 cycles/elem along
                # the FREE dim, so [128,4] (0.2us) beats [.,512] (3us)
                po2 = (h % 2) * 64
                uS = rpool.tile([64, 512], f32, tag="uS")
                nc.vector.tensor_copy(out=uS, in_=po_t[0:HD, :])
                dS = rpool.tile([1, 512], f32, tag="dS")
                nc.vector.tensor_copy(out=dS, in_=po_t[HD : HD + 1, :])
                scr = ps_mm.tile([128, 512], f32, tag="mm")
                dT = scr[:, 0:4]
                rrow = scr[0:1, 0:512]
                for c in range(KC):
                    nc.tensor.transpose(
                        dT[:, c : c + 1], dS[:, c * 128 : (c + 1) * 128],
                        ident[0:1, 0:1],
                    )
                rT = small.tile([128, 4], f32, tag="rT")
                nc.vector.reciprocal(rT, dT)
                for c in range(KC):
                    nc.tensor.transpose(
                        rrow[:, c * 128 : (c + 1) * 128], rT[:, c : c + 1], ident
                    )
                recip_s = rpool.tile([1, 512], f32, tag="recip")
                nc.vector.tensor_copy(out=recip_s, in_=rrow)
                rb = rpool.tile([64, 512], f32, tag="rb")
                nc.gpsimd.partition_broadcast(rb, recip_s, channels=64)
                nc.gpsimd.tensor_mul(oT[po2 : po2 + 64, t, nsl], uS, rb)

            # one continuous stream over all (pair, chunk, group) units; the
            # AV matmuls run one unit behind the score/exp/mask front so the
            # in-order PE queue never blocks the exp stream, and the pipeline
            # never drains at pair boundaries
            pend = {}   # live pair state: (t, ncc) -> (poE, poO, nsl)
            pts = {}    # unit -> (ptA, ptB)

            def emit_av(unit):
                t, ncc, g = unit
                poE, poO, _ = pend[(t, ncc)]
                slotE = slice((2 * t) * HW, (2 * t + 1) * HW)
                slotO = slice((2 * t + 1) * HW, (2 * t + 2) * HW)
                ptA, ptB = pts.pop(unit)
                for u in range(2):
                    j = 2 * g + u
                    usl = slice(u * 512, (u + 1) * 512)
                    nc.tensor.matmul(
                        poE, lhsT=vS[:, j, slotE], rhs=ptA[:, usl],
                        start=(j == 0), stop=(j == MT - 1),
                    )
                    nc.tensor.matmul(
                        poO, lhsT=vS[:, j, slotO], rhs=ptB[:, usl],
                        start=(j == 0), stop=(j == MT - 1),
                    )
                if g == MG - 1:
                    poE, poO, nsl = pend.pop((t, ncc))
                    norm = normalize_flat if (ncc == 1 and t >= 2) else normalize
                    norm(poE, 2 * t, t, nsl)
                    norm(poO, 2 * t + 1, t, nsl)

            def attend_all(pair_order, fillmap):
                units = [(t, ncc, g) for (t, ncc) in pair_order for g in range(MG)]
                prev = None
                for unit in units:
                    t, ncc, g = unit
                    nsl = slice(ncc * 512, (ncc + 1) * 512)
                    if g == 0:
                        poE_new = ps_o.tile([HW, 512], f32, tag="poE")
                        poO_new = ps_o.tile([HW, 512], f32, tag="poO")
                        pend[(t, ncc)] = (poE_new, poO_new, nsl)
                    poE, poO, _ = pend[(t, ncc)]
                    psA = ps_sA.tile([128, 1024], f32, tag="sA")
                    psB = ps_sB.tile([128, 1024], f32, tag="sB")
                    for u in range(2):
                        j = 2 * g + u
                        usl = slice(u * 512, (u + 1) * 512)
                        # two heads on disjoint PE row halves -> concurrent
                        nc.tensor.matmul(
                            psA[:, usl],
                            lhsT=kT[0:64, t, j * 128 : (j + 1) * 128],
                            rhs=qT[0:64, t, nsl],
                            start=True, stop=True,
                        )
                        nc.tensor.matmul(
                            psB[:, usl],
                            lhsT=kT[64:128, t, j * 128 : (j + 1) * 128],
                            rhs=qT[64:128, t, nsl],
                            start=True, stop=True,
                        )
                    ptA = ppool.tile([128, 1024], bf16, tag="pt")
                    nc.scalar.activation(ptA, psA, Exp)
                    ptB = ppool.tile([128, 1024], bf16, tag="pt")
                    nc.scalar.activation(ptB, psB, Exp)
                    nc.vector.tensor_mul(ptA, ptA, maskS[:, ncc, g, :])
                    nc.vector.tensor_mul(ptB, ptB, maskS[:, ncc, g, :])
                    pts[unit] = (ptA, ptB)
                    for f in fillmap.get((t, ncc), {}).get(g, ()):
                        f()
                    if prev is not None:
                        emit_av(prev)
                    prev = unit
                emit_av(prev)

            # ---- output projection + residual + LayerNorm ----------------
            qres_r = qres_d[:].rearrange("(t p) d -> p t d", p=128)
            out_r = out_d[:].rearrange("(t p) d -> p t d", p=128)
            ot_state = {}

            def out_front(nt):
                ps = ps_mm.tile([128, 512], f32, tag="mm")
                for a in range(KC):
                    nc.tensor.matmul(
                        ps,
                        lhsT=oT[:, a, nt * 128 : (nt + 1) * 128],
                        rhs=wo[:, a, :],
                        start=(a == 0),
                        stop=(a == KC - 1),
                    )
                qres_t = ypool.tile([128, D], f32, tag="qres")
                nc.sync.dma_start(out=qres_t, in_=qres_r[:, nt, :])
                x_t = xpool.tile([128, D], f32, tag="x")
                if K_XT:
                    nc.gpsimd.tensor_add(x_t, ps, qres_t)
                else:
                    nc.vector.tensor_add(x_t, ps, qres_t)
                stats = small.tile([128, 6], f32, tag="stats")
                nc.vector.bn_stats(out=stats, in_=x_t)
                mv = mvpool.tile([128, 2], f32, tag="mv")
                nc.vector.bn_aggr(out=mv, in_=stats)
                ot_state[nt] = (x_t, mv)

            rstd_store = {}

            def rstd_batch(nts):
                # one Sqrt activation for a wave of tiles -> 2 ACT table
                # switches per wave instead of 2 per tile
                vcol = small.tile([128, 4], f32, tag="vcol")
                for i, nt in enumerate(nts):
                    nc.vector.tensor_copy(out=vcol[:, i : i + 1],
                                          in_=ot_state[nt][1][:, 1:2])
                sd = small.tile([128, 4], f32, tag="sd")
                nc.scalar.activation(sd, vcol, Sqrt, bias=eps_t)
                rs = mvpool.tile([128, 4], f32, tag="rs")
                nc.vector.reciprocal(rs, sd)
                for i, nt in enumerate(nts):
                    rstd_store[nt] = (rs, i)

            def out_back(nt, tail=False):
                x_t, mv = ot_state.pop(nt)
                rs, i = rstd_store.pop(nt)
                xn = ypool.tile([128, D], f32, tag="xn")
                nc.vector.tensor_scalar(
                    out=xn, in0=x_t, scalar1=mv[:, 0:1], scalar2=rs[:, i : i + 1],
                    op0=sub, op1=mult,
                )
                y_t = ypool.tile([128, D], f32, tag="y")
                if tail:  # DVE is free at the tail; gpsimd mul is slower
                    nc.vector.tensor_mul(y_t, xn, gamma_b)
                    nc.vector.tensor_add(y_t, y_t, beta_b)
                else:
                    nc.gpsimd.tensor_mul(y_t, xn, gamma_b)
                    nc.gpsimd.tensor_add(y_t, y_t, beta_b)
                nc.sync.dma_start(out=out_r[:, nt, :], in_=y_t)

            # ---- emission schedule ---------------------------------------
            # ramp: just enough projection work for pair 0 + first AV tiles
            q_proj(0, 0)
            q_proj(0, 1)
            for mc in range(MCH):
                k_proj(0, mc)
            v_proj(0)
            v_proj(1)

            def C(f, *a):
                return lambda: f(*a)

            # pair-0 fillers: V tiles JIT (AV of group g needs v(2g,2g+1);
            # slot g supplies v(2g+2,2g+3)); pair-p prereqs (qT/kT complete)
            # must be emitted before pair p starts
            f00 = {
                0: (C(v_proj, 2), C(v_proj, 3)),
                1: (C(v_proj, 4), C(v_proj, 5)),
                2: (C(v_proj, 6), C(v_proj, 7)),
                3: (C(v_proj, 8), C(v_proj, 9)),
                4: (C(v_proj, 10), C(v_proj, 11)),
                5: (C(v_proj, 12), C(v_proj, 13)),
                6: (C(v_proj, 14), C(v_proj, 15), C(q_proj, 1, 0)),
                7: (C(q_proj, 1, 1), C(k_proj, 1, 0)),
            }
            # k(t,mc) feeds score groups 2mc..2mc+1 of pair t: later chunks
            # can trail into pair t itself as long as they stay 2 groups ahead
            f10 = {
                0: (C(k_proj, 1, 1),),
                1: (C(k_proj, 1, 2), C(k_proj, 1, 3)),
                3: (C(q_proj, 2, 0),),
                4: (C(q_proj, 2, 1),),
                5: (C(k_proj, 2, 0),),
                6: (C(k_proj, 2, 1),),
                7: (C(k_proj, 2, 2), C(k_proj, 2, 3)),
            }
            f20 = {
                0: (C(q_proj, 3, 0),),
                1: (C(q_proj, 3, 1),),
                4: (C(k_proj, 3, 0),),
                5: (C(k_proj, 3, 1),),
                6: (C(k_proj, 3, 2), C(k_proj, 3, 3)),
            }
            # Scalar queue is strict FIFO: the wave-A Sqrt must enter it only
            # when its bn-stats deps are long done, else every later exp
            # stalls behind it.  fronts 0-3 early in ncc1, Sqrt a full pair
            # later, backs on the last pair.
            f01 = {2: (C(out_front, 0),), 4: (C(out_front, 1),),
                   6: (C(out_front, 2),)}
            f11 = {0: (C(out_front, 3),)}
            f21 = {4: (C(rstd_batch, (0, 1, 2, 3)),)}
            f31 = {0: (C(out_back, 0),), 2: (C(out_back, 1),),
                   4: (C(out_back, 2),), 6: (C(out_back, 3),)}

            pair_order = [(0, 0), (1, 0), (2, 0), (3, 0),
                          (0, 1), (1, 1), (2, 1), (3, 1)]
            fillmap = {(0, 0): f00, (1, 0): f10, (2, 0): f20,
                       (0, 1): f01, (1, 1): f11, (2, 1): f21, (3, 1): f31}
            attend_all(pair_order, fillmap)
            out_front(4)
            out_front(5)
            out_front(6)
            out_front(7)
            rstd_batch((4, 5, 6, 7))
            out_back(4, tail=True)
            out_back(5, tail=True)
            out_back(6, tail=True)
            out_back(7, tail=True)

    nc.compile()
    return nc


def kernel(**inputs):
    from concourse.bass_utils import run_bass_kernel_spmd

    if "nc" not in _CACHE:
        _CACHE["nc"] = _build()
    nc = _CACHE["nc"]

    query = np.asarray(inputs["query"], dtype=np.float32)
    key = np.asarray(inputs["key"], dtype=np.float32)
    value = np.asarray(inputs["value"], dtype=np.float32)
    mask = np.asarray(inputs["mask"])
    WQ = np.asarray(inputs["WQ"], dtype=np.float32)
    WK = np.asarray(inputs["WK"], dtype=np.float32)
    WV = np.asarray(inputs["WV"], dtype=np.float32)
    WO = np.asarray(inputs["WO"], dtype=np.float32)
    bO = np.asarray(inputs["bO"], dtype=np.float32)
    gamma = np.asarray(inputs["gamma"], dtype=np.float32)
    beta = np.asarray(inputs["beta"], dtype=np.float32)

    scale = np.float32(1.0 / np.sqrt(HD))
    wqT = np.ascontiguousarray(WQ.T * scale).astype(BF16)
    wkT = np.ascontiguousarray(WK.T).astype(BF16)
    wvT = np.ascontiguousarray(WV.T).astype(BF16)
    woT = np.ascontiguousarray(WO.T).astype(BF16)
    gamma_in = gamma.reshape(1, D)
    beta_in = beta.reshape(1, D)
    mask_bin = (mask != 0)

    in_maps = []
    for c in range(NCORES):
        b, n0 = c // 2, (c % 2) * NS
        # mask, transposed and prepacked per (n-chunk, score-group):
        # maskP[ncc, g, p, u*512+nn] = maskT[g*256+u*128+p, ncc*512+nn]
        mT = np.ascontiguousarray(mask_bin[b, n0 : n0 + NS, :].T)  # [M, NS]
        mP = (
            mT.reshape(8, 2, 128, 2, 512)
            .transpose(3, 0, 2, 1, 4)
            .reshape(2 * 8 * 128, 1024)
        )
        in_maps.append({
            "xqT": np.ascontiguousarray(query[b, n0 : n0 + NS, :].T).astype(BF16),
            "xkT": np.ascontiguousarray(key[b].T).astype(BF16),
            "xvT": np.ascontiguousarray(value[b].T).astype(BF16),
            "maskP": np.ascontiguousarray(mP).astype(BF16),
            "qres": np.ascontiguousarray(query[b, n0 : n0 + NS, :] + bO[None, :]),
            "wqT": wqT, "wkT": wkT, "wvT": wvT, "woT": woT,
            "gamma": gamma_in, "beta": beta_in,
        })

    trace = bool(int(os.environ.get("BASS_KERNEL_TRACE", "0")))
    res = run_bass_kernel_spmd(nc, in_maps, core_ids=list(range(NCORES)), trace=trace)
    _CACHE["last_results"] = res

    out = np.empty((B, N, D), dtype=np.float32)
    for c in range(NCORES):
        b, n0 = c // 2, (c % 2) * NS
        out[b, n0 : n0 + NS, :] = res.results[c]["out"]
    return out


# revision 30
# speedup vs baseline: 1.3315x; 1.3026x over previous
"""MultiHeadGraphAttention TRN2 kernel, v2.

Data-parallel over (batch, query-half): core c handles batch c//2, query rows
(c%2)*1024 .. +1024.  All matmuls bf16 (fp32 PSUM); softmax + LayerNorm fp32.

v2 changes vs baseline (337us):
 - ScalarE is the wall (~130us of exp).  Everything else is arranged to hide
   under it: PSUM->SBUF projection copies moved to DVE, LayerNorm rstd uses
   ln+exp (both in the natural_log_exp_and_others table set -> no table
   thrash; Sqrt previously forced 10 table reloads mid-kernel and stalled the
   exp stream).
 - Score matmuls of a head PAIR run concurrently on disjoint PE row halves
   (K=64 each; tile_position auto-derived from base partitions 0/64).
 - Attention inner loop is software-pipelined: AV matmuls of group g-1 are
   emitted after the score matmuls of group g, so the in-order PE queue never
   blocks the next score tile (and the exp stream) behind a mask-waiting AV.
 - Input DMAs are split per consumption chunk and emitted in consumption
   order; projections start as soon as their inputs land (~4us) instead of
   after all input DMA (~38us).  Remaining projections are threaded into the
   attention stream as PE filler so the PE never idles > ~1us (HAM stays at
   K=8/8).
 - softmax denominator from an appended ones-column on V (row 64 of the AV
   output); reciprocal on DVE, partition-broadcast + normalize mul on GPSIMD.
"""

import os
import sys

import numpy as np

try:
    import concourse  # noqa: F401
except ImportError:  # harness runs from a bare dir; the repo is a fixed path
    sys.path.insert(0, "/opt/trn_rl_repo")

import ml_dtypes

B, N, M, D, H, HD = 4, 2048, 2048, 512, 8, 64
NS = 1024          # query rows per core
NCORES = 8
LN_EPS = 1e-5
BF16 = ml_dtypes.bfloat16

_CACHE = {}

# fallback knobs (read once at build)
# NOTE: reciprocal_approx_fast passes CoreSim but returns garbage on HW.
# NOTE: GPSIMD cannot access PSUM (BIR verifier) -> PSUM-reading ops on DVE.
K_XT = int(os.environ.get("K_XT", "0"))   # x_t add on gpsimd vs vector


def _build(ln_affine=True):
    import concourse.bass as bass  # noqa: F401
    import concourse.tile as tile
    from concourse import bacc, mybir
    from concourse.masks import make_identity

    f32 = mybir.dt.float32
    bf16 = mybir.dt.bfloat16
    Exp = mybir.ActivationFunctionType.Exp
    Sqrt = mybir.ActivationFunctionType.Sqrt
    sub = mybir.AluOpType.subtract
    mult = mybir.AluOpType.mult

    nc = bacc.Bacc(None, target_bir_lowering=False, debug=False)

    xqT_d = nc.dram_tensor("xqT", [D, NS], bf16, kind="ExternalInput")
    xkT_d = nc.dram_tensor("xkT", [D, M], bf16, kind="ExternalInput")
    xvT_d = nc.dram_tensor("xvT", [D, M], bf16, kind="ExternalInput")
    maskP_d = nc.dram_tensor("maskP", [2 * 8 * 128, 1024], bf16, kind="ExternalInput")
    qres_d = nc.dram_tensor("qres", [NS, D], f32, kind="ExternalInput")
    wqT_d = nc.dram_tensor("wqT", [D, D], bf16, kind="ExternalInput")
    wkT_d = nc.dram_tensor("wkT", [D, D], bf16, kind="ExternalInput")
    wvT_d = nc.dram_tensor("wvT", [D, D], bf16, kind="ExternalInput")
    woT_d = nc.dram_tensor("woT", [D, D], bf16, kind="ExternalInput")
    gamma_d = nc.dram_tensor("gamma", [1, D], f32, kind="ExternalInput")
    beta_d = nc.dram_tensor("beta", [1, D], f32, kind="ExternalInput")
    out_d = nc.dram_tensor("out", [NS, D], f32, kind="ExternalOutput")

    KC = D // 128      # 4 contraction chunks of 128
    NCH = NS // 512    # 2 query-column chunks
    MT = M // 128      # 16 key-position tiles
    MCH = M // 512     # 4 key chunks of 512
    MG = MT // 2       # 8 score groups (2 key tiles per group)
    HW = HD + 1        # per-head V slot width (64 V cols + ones col)

    with tile.TileContext(nc) as tc:
        with (
            tc.tile_pool(name="big", bufs=1) as big,
            tc.tile_pool(name="wpool", bufs=1) as wpool,
            tc.tile_pool(name="ppool", bufs=4) as ppool,
            tc.tile_pool(name="xpool", bufs=5) as xpool,
            tc.tile_pool(name="mvpool", bufs=6) as mvpool,
            tc.tile_pool(name="ypool", bufs=3) as ypool,
            tc.tile_pool(name="rpool", bufs=2) as rpool,
            tc.tile_pool(name="small", bufs=6) as small,
            tc.tile_pool(name="ps_mm", bufs=2, space="PSUM") as ps_mm,
            tc.tile_pool(name="ps_sA", bufs=1, space="PSUM") as ps_sA,
            tc.tile_pool(name="ps_sB", bufs=1, space="PSUM") as ps_sB,
            tc.tile_pool(name="ps_o", bufs=1, space="PSUM") as ps_o,
        ):
            # ---- resident SBUF tensors -----------------------------------
            xqT = big.tile([128, KC, NS], bf16, tag="xqT")
            xkT = big.tile([128, KC, M], bf16, tag="xkT")
            xvT = big.tile([128, KC, M], bf16, tag="xvT")
            maskS = big.tile([128, NCH, MG, 1024], bf16, tag="maskS")
            qT = big.tile([128, KC, NS], bf16, tag="qT")
            kT = big.tile([128, KC, M], bf16, tag="kT")
            vS = big.tile([128, MT, H * HW], bf16, tag="vS")
            oT = big.tile([128, KC, NS], bf16, tag="oT")
            wq = wpool.tile([128, KC, D], bf16, tag="wq")
            wk = wpool.tile([128, KC, D], bf16, tag="wk")
            wv = wpool.tile([128, KC, D], bf16, tag="wv")
            wo = wpool.tile([128, KC, D], bf16, tag="wo")
            gamma_b = wpool.tile([128, D], f32, tag="gamma_b")
            beta_b = wpool.tile([128, D], f32, tag="beta_b")
            gamma_1 = wpool.tile([1, D], f32, tag="gamma_1")
            beta_1 = wpool.tile([1, D], f32, tag="beta_1")
            eps_t = wpool.tile([128, 1], f32, tag="eps")
            ident = wpool.tile([128, 128], f32, tag="ident")
            make_identity(nc, ident)

            # ---- setup (no DMA dependencies; engines idle early) ---------
            nc.vector.memset(eps_t, LN_EPS)
            # ones column per head in the augmented V (softmax denominator
            # lands as row 64 of the AV matmul output)
            nc.vector.memset(
                vS[:].rearrange("p j (h x) -> p j h x", x=HW)[:, :, :, HD : HD + 1],
                1.0,
            )

            # ---- input DMAs, split per consumption chunk, priority order -
            xq_r = xqT_d[:].rearrange("(c p) n -> p c n", p=128)
            xk_r = xkT_d[:].rearrange("(c p) n -> p c n", p=128)
            xv_r = xvT_d[:].rearrange("(c p) n -> p c n", p=128)
            mk_r = maskP_d[:].rearrange("(c g p) n -> p c g n", c=NCH, g=MG)

            nc.sync.dma_start(out=wq, in_=wqT_d[:].rearrange("(c p) o -> p c o", p=128))
            for ncc in range(NCH):
                sl = slice(ncc * 512, (ncc + 1) * 512)
                nc.sync.dma_start(out=xqT[:, :, sl], in_=xq_r[:, :, sl])
            nc.sync.dma_start(out=wk, in_=wkT_d[:].rearrange("(c p) o -> p c o", p=128))
            for mc in range(MCH):
                sl = slice(mc * 512, (mc + 1) * 512)
                nc.sync.dma_start(out=xkT[:, :, sl], in_=xk_r[:, :, sl])
            nc.sync.dma_start(out=maskS[:, 0, 0, :], in_=mk_r[:, 0, 0, :])
            nc.sync.dma_start(out=maskS[:, 0, 1, :], in_=mk_r[:, 0, 1, :])
            nc.sync.dma_start(out=wv, in_=wvT_d[:].rearrange("(c p) o -> p c o", p=128))
            for jc in range(4):
                sl = slice(jc * 256, (jc + 1) * 256)
                nc.sync.dma_start(out=xvT[:, :, sl], in_=xv_r[:, :, sl])
            nc.sync.dma_start(out=maskS[:, 0, 2, :], in_=mk_r[:, 0, 2, :])
            nc.sync.dma_start(out=maskS[:, 0, 3, :], in_=mk_r[:, 0, 3, :])
            for jc in range(4, 8):
                sl = slice(jc * 256, (jc + 1) * 256)
                nc.sync.dma_start(out=xvT[:, :, sl], in_=xv_r[:, :, sl])
            for g in range(4, MG):
                nc.sync.dma_start(out=maskS[:, 0, g, :], in_=mk_r[:, 0, g, :])
            nc.sync.dma_start(out=wo, in_=woT_d[:].rearrange("(c p) o -> p c o", p=128))
            for g in range(MG):
                nc.sync.dma_start(out=maskS[:, 1, g, :], in_=mk_r[:, 1, g, :])
            nc.sync.dma_start(out=gamma_1, in_=gamma_d[:])
            nc.sync.dma_start(out=beta_1, in_=beta_d[:])
            nc.gpsimd.partition_broadcast(gamma_b, gamma_1, channels=128)
            nc.gpsimd.partition_broadcast(beta_b, beta_1, channels=128)

            # ---- projection emitters (PSUM->SBUF copies on DVE) ----------
            def q_proj(t, ncc):
                ps = ps_mm.tile([128, 512], f32, tag="mm")
                sl = slice(ncc * 512, (ncc + 1) * 512)
                for kc in range(KC):
                    nc.tensor.matmul(
                        ps,
                        lhsT=wq[:, kc, t * 128 : (t + 1) * 128],
                        rhs=xqT[:, kc, sl],
                        start=(kc == 0),
                        stop=(kc == KC - 1),
                    )
                nc.vector.tensor_copy(out=qT[:, t, sl], in_=ps)

            def k_proj(t, mc):
                ps = ps_mm.tile([128, 512], f32, tag="mm")
                sl = slice(mc * 512, (mc + 1) * 512)
                for kc in range(KC):
                    nc.tensor.matmul(
                        ps,
                        lhsT=wk[:, kc, t * 128 : (t + 1) * 128],
                        rhs=xkT[:, kc, sl],
                        start=(kc == 0),
                        stop=(kc == KC - 1),
                    )
                nc.vector.tensor_copy(out=kT[:, t, sl], in_=ps)

            def v_proj(j):
                # V[m, o] straight, scattered into per-head 65-wide slots
                ps = ps_mm.tile([128, 512], f32, tag="mm")
                for kc in range(KC):
                    nc.tensor.matmul(
                        ps,
                        lhsT=xvT[:, kc, j * 128 : (j + 1) * 128],
                        rhs=wv[:, kc, :],
                        start=(kc == 0),
                        stop=(kc == KC - 1),
                    )
                nc.vector.tensor_copy(
                    out=vS[:, j, :].rearrange("p (h x) -> p h x", x=HW)[:, :, 0:HD],
                    in_=ps[:].rearrange("p (h x) -> p h x", x=HD),
                )

            # ---- attention: head pair 2t/2t+1, software-pipelined --------
            # GPSIMD ucode note: partition_broadcast and tensor ops live in
            # DIFFERENT gpsimd libraries; alternating them costs a ~5us
            # UNLOAD_LIB/LOAD_LIB pair each time.  GPSIMD therefore runs
            # ONLY partition_broadcast; every tensor op goes to DVE.
            def normalize(po_t, h, t, nsl):
                # reciprocal via the PE-transpose dance — DVE reciprocal is
                # ~6 cycles/elem along the FREE dim, so [128,4] (0.2us)
                # beats [1,512] (3us)
                po2 = (h % 2) * 64
                dS = rpool.tile([1, 512], f32, tag="dS")
                nc.vector.tensor_copy(out=dS, in_=po_t[HD : HD + 1, :])
                scr = ps_mm.tile([128, 512], f32, tag="mm")
                dT = scr[:, 0:4]
                rrow = scr[0:1, 0:512]
                for c in range(KC):
                    nc.tensor.transpose(
                        dT[:, c : c + 1], dS[:, c * 128 : (c + 1) * 128],
                        ident[0:1, 0:1],
                    )
                rT = small.tile([128, 4], f32, tag="rT")
                nc.vector.reciprocal(rT, dT)
                for c in range(KC):
                    nc.tensor.transpose(
                        rrow[:, c * 128 : (c + 1) * 128], rT[:, c : c + 1], ident
                    )
                recip_s = rpool.tile([1, 512], f32, tag="recip")
                nc.vector.tensor_copy(out=recip_s, in_=rrow)
                rb = rpool.tile([64, 512], f32, tag="rb")
                nc.gpsimd.partition_broadcast(rb, recip_s, channels=64)
                nc.vector.tensor_mul(oT[po2 : po2 + 64, t, nsl], po_t[0:HD, :], rb)

            # one continuous stream over all (pair, chunk, group) units; the
            # AV matmuls run one unit behind the score/exp/mask front so the
            # in-order PE queue never blocks the exp stream, and the pipeline
            # never drains at pair boundaries
            pend = {}   # live pair state: (t, ncc) -> (poE, poO, nsl)
            pts = {}    # unit -> (ptA, ptB)

            def emit_av(unit):
                t, ncc, g = unit
                poE, poO, _ = pend[(t, ncc)]
                slotE = slice((2 * t) * HW, (2 * t + 1) * HW)
                slotO = slice((2 * t + 1) * HW, (2 * t + 2) * HW)
                ptA, ptB = pts.pop(unit)
                for u in range(2):
                    j = 2 * g + u
                    usl = slice(u * 512, (u + 1) * 512)
                    nc.tensor.matmul(
                        poE, lhsT=vS[:, j, slotE], rhs=ptA[:, usl],
                        start=(j == 0), stop=(j == MT - 1),
                    )
                    nc.tensor.matmul(
                        poO, lhsT=vS[:, j, slotO], rhs=ptB[:, usl],
                        start=(j == 0), stop=(j == MT - 1),
                    )
                if g == MG - 1:
                    poE, poO, nsl = pend.pop((t, ncc))
                    normalize(poE, 2 * t, t, nsl)
                    normalize(poO, 2 * t + 1, t, nsl)

            def attend_all(pair_order, fillmap):
                units = [(t, ncc, g) for (t, ncc) in pair_order for g in range(MG)]
                prev = None
                for unit in units:
                    t, ncc, g = unit
                    nsl = slice(ncc * 512, (ncc + 1) * 512)
                    if g == 0:
                        poE_new = ps_o.tile([HW, 512], f32, tag="poE")
                        poO_new = ps_o.tile([HW, 512], f32, tag="poO")
                        pend[(t, ncc)] = (poE_new, poO_new, nsl)
                    poE, poO, _ = pend[(t, ncc)]
                    psA = ps_sA.tile([128, 1024], f32, tag="sA")
                    psB = ps_sB.tile([128, 1024], f32, tag="sB")
                    for u in range(2):
                        j = 2 * g + u
                        usl = slice(u * 512, (u + 1) * 512)
                        # two heads on disjoint PE row halves -> concurrent
                        nc.tensor.matmul(
                            psA[:, usl],
                            lhsT=kT[0:64, t, j * 128 : (j + 1) * 128],
                            rhs=qT[0:64, t, nsl],
                            start=True, stop=True,
                        )
                        nc.tensor.matmul(
                            psB[:, usl],
                            lhsT=kT[64:128, t, j * 128 : (j + 1) * 128],
                            rhs=qT[64:128, t, nsl],
                            start=True, stop=True,
                        )
                    ptA = ppool.tile([128, 1024], bf16, tag="pt")
                    nc.scalar.activation(ptA, psA, Exp)
                    ptB = ppool.tile([128, 1024], bf16, tag="pt")
                    nc.scalar.activation(ptB, psB, Exp)
                    nc.vector.tensor_mul(ptA, ptA, maskS[:, ncc, g, :])
                    nc.vector.tensor_mul(ptB, ptB, maskS[:, ncc, g, :])
                    pts[unit] = (ptA, ptB)
                    for f in fillmap.get((t, ncc), {}).get(g, ()):
                        f()
                    if prev is not None:
                        emit_av(prev)
                    prev = unit
                emit_av(prev)

            # ---- output projection + residual + LayerNorm ----------------
            qres_r = qres_d[:].rearrange("(t p) d -> p t d", p=128)
            out_r = out_d[:].rearrange("(t p) d -> p t d", p=128)
            ot_state = {}

            def out_front(nt):
                ps = ps_mm.tile([128, 512], f32, tag="mm")
                for a in range(KC):
                    nc.tensor.matmul(
                        ps,
                        lhsT=oT[:, a, nt * 128 : (nt + 1) * 128],
                        rhs=wo[:, a, :],
                        start=(a == 0),
                        stop=(a == KC - 1),
                    )
                qres_t = ypool.tile([128, D], f32, tag="qres")
                nc.sync.dma_start(out=qres_t, in_=qres_r[:, nt, :])
                x_t = xpool.tile([128, D], f32, tag="x")
                if K_XT:
                    nc.gpsimd.tensor_add(x_t, ps, qres_t)
                else:
                    nc.vector.tensor_add(x_t, ps, qres_t)
                stats = small.tile([128, 6], f32, tag="stats")
                nc.vector.bn_stats(out=stats, in_=x_t)
                mv = mvpool.tile([128, 2], f32, tag="mv")
                nc.vector.bn_aggr(out=mv, in_=stats)
                ot_state[nt] = (x_t, mv)

            rstd_store = {}

            def rstd_batch(nts):
                # one Sqrt activation for a wave of tiles -> 2 ACT table
                # switches per wave instead of 2 per tile
                vcol = small.tile([128, 4], f32, tag="vcol")
                for i, nt in enumerate(nts):
                    nc.vector.tensor_copy(out=vcol[:, i : i + 1],
                                          in_=ot_state[nt][1][:, 1:2])
                sd = small.tile([128, 4], f32, tag="sd")
                nc.scalar.activation(sd, vcol, Sqrt, bias=eps_t)
                rs = mvpool.tile([128, 4], f32, tag="rs")
                nc.vector.reciprocal(rs, sd)
                for i, nt in enumerate(nts):
                    rstd_store[nt] = (rs, i)

            def out_back(nt, tail=False):
                x_t, mv = ot_state.pop(nt)
                rs, i = rstd_store.pop(nt)
                xn = ypool.tile([128, D], f32, tag="xn")
                nc.vector.tensor_scalar(
                    out=xn, in0=x_t, scalar1=mv[:, 0:1], scalar2=rs[:, i : i + 1],
                    op0=sub, op1=mult,
                )
                if ln_affine:  # on DVE: gpsimd is reserved for broadcasts
                    y_t = ypool.tile([128, D], f32, tag="y")
                    nc.vector.tensor_mul(y_t, xn, gamma_b)
                    nc.vector.tensor_add(y_t, y_t, beta_b)
                else:          # gamma==1, beta==0 (checked host-side)
                    y_t = xn
                nc.sync.dma_start(out=out_r[:, nt, :], in_=y_t)

            # ---- emission schedule ---------------------------------------
            # ramp: just enough projection work for pair 0 + first AV tiles
            q_proj(0, 0)
            q_proj(0, 1)
            for mc in range(MCH):
                k_proj(0, mc)
            v_proj(0)
            v_proj(1)

            def C(f, *a):
                return lambda: f(*a)

            # pair-0 fillers: V tiles JIT (AV of group g needs v(2g,2g+1);
            # slot g supplies v(2g+2,2g+3)); pair-p prereqs (qT/kT complete)
            # must be emitted before pair p starts
            f00 = {
                0: (C(v_proj, 2), C(v_proj, 3)),
                1: (C(v_proj, 4), C(v_proj, 5)),
                2: (C(v_proj, 6), C(v_proj, 7)),
                3: (C(v_proj, 8), C(v_proj, 9)),
                4: (C(v_proj, 10), C(v_proj, 11)),
                5: (C(v_proj, 12), C(v_proj, 13)),
                6: (C(v_proj, 14), C(v_proj, 15), C(q_proj, 1, 0)),
                7: (C(q_proj, 1, 1), C(k_proj, 1, 0)),
            }
            # k(t,mc) feeds score groups 2mc..2mc+1 of pair t: later chunks
            # can trail into pair t itself as long as they stay 2 groups ahead
            f10 = {
                0: (C(k_proj, 1, 1),),
                1: (C(k_proj, 1, 2), C(k_proj, 1, 3)),
                3: (C(q_proj, 2, 0),),
                4: (C(q_proj, 2, 1),),
                5: (C(k_proj, 2, 0),),
                6: (C(k_proj, 2, 1),),
                7: (C(k_proj, 2, 2), C(k_proj, 2, 3)),
            }
            f20 = {
                0: (C(q_proj, 3, 0),),
                1: (C(q_proj, 3, 1),),
                4: (C(k_proj, 3, 0),),
                5: (C(k_proj, 3, 1),),
                6: (C(k_proj, 3, 2), C(k_proj, 3, 3)),
            }
            # Scalar queue is strict FIFO: the wave-A Sqrt must enter it only
            # when its bn-stats deps are long done, else every later exp
            # stalls behind it.  fronts 0-3 early in ncc1, Sqrt a full pair
            # later, backs on the last pair.
            f01 = {2: (C(out_front, 0),), 4: (C(out_front, 1),),
                   6: (C(out_front, 2),)}
            f11 = {0: (C(out_front, 3),)}
            f21 = {4: (C(rstd_batch, (0, 1, 2, 3)),)}
            f31 = {0: (C(out_back, 0),), 2: (C(out_back, 1),),
                   4: (C(out_back, 2),), 6: (C(out_back, 3),)}

            pair_order = [(0, 0), (1, 0), (2, 0), (3, 0),
                          (0, 1), (1, 1), (2, 1), (3, 1)]
            fillmap = {(0, 0): f00, (1, 0): f10, (2, 0): f20,
                       (0, 1): f01, (1, 1): f11, (2, 1): f21, (3, 1): f31}
            attend_all(pair_order, fillmap)
            out_front(4)
            out_front(5)
            out_front(6)
            out_front(7)
            rstd_batch((4, 5, 6, 7))
            out_back(4, tail=True)
            out_back(5, tail=True)
            out_back(6, tail=True)
            out_back(7, tail=True)

    nc.compile()
    return nc


def kernel(**inputs):
    from concourse.bass_utils import run_bass_kernel_spmd

    gamma_a = np.asarray(inputs["gamma"], dtype=np.float32)
    beta_a = np.asarray(inputs["beta"], dtype=np.float32)
    ln_affine = bool(np.any(gamma_a != 1.0) or np.any(beta_a != 0.0))
    ck = ("nc", ln_affine)
    if ck not in _CACHE:
        _CACHE[ck] = _build(ln_affine)
    nc = _CACHE[ck]

    query = np.asarray(inputs["query"], dtype=np.float32)
    key = np.asarray(inputs["key"], dtype=np.float32)
    value = np.asarray(inputs["value"], dtype=np.float32)
    mask = np.asarray(inputs["mask"])
    WQ = np.asarray(inputs["WQ"], dtype=np.float32)
    WK = np.asarray(inputs["WK"], dtype=np.float32)
    WV = np.asarray(inputs["WV"], dtype=np.float32)
    WO = np.asarray(inputs["WO"], dtype=np.float32)
    bO = np.asarray(inputs["bO"], dtype=np.float32)
    gamma = np.asarray(inputs["gamma"], dtype=np.float32)
    beta = np.asarray(inputs["beta"], dtype=np.float32)

    scale = np.float32(1.0 / np.sqrt(HD))
    wqT = np.ascontiguousarray(WQ.T * scale).astype(BF16)
    wkT = np.ascontiguousarray(WK.T).astype(BF16)
    wvT = np.ascontiguousarray(WV.T).astype(BF16)
    woT = np.ascontiguousarray(WO.T).astype(BF16)
    gamma_in = gamma.reshape(1, D)
    beta_in = beta.reshape(1, D)
    mask_bin = (mask != 0)

    in_maps = []
    for c in range(NCORES):
        b, n0 = c // 2, (c % 2) * NS
        # mask, transposed and prepacked per (n-chunk, score-group):
        # maskP[ncc, g, p, u*512+nn] = maskT[g*256+u*128+p, ncc*512+nn]
        mT = np.ascontiguousarray(mask_bin[b, n0 : n0 + NS, :].T)  # [M, NS]
        mP = (
            mT.reshape(8, 2, 128, 2, 512)
            .transpose(3, 0, 2, 1, 4)
            .reshape(2 * 8 * 128, 1024)
        )
        in_maps.append({
            "xqT": np.ascontiguousarray(query[b, n0 : n0 + NS, :].T).astype(BF16),
            "xkT": np.ascontiguousarray(key[b].T).astype(BF16),
            "xvT": np.ascontiguousarray(value[b].T).astype(BF16),
            "maskP": np.ascontiguousarray(mP).astype(BF16),
            "qres": np.ascontiguousarray(query[b, n0 : n0 + NS, :] + bO[None, :]),
            "wqT": wqT, "wkT": wkT, "wvT": wvT, "woT": woT,
            "gamma": gamma_in, "beta": beta_in,
        })

    trace = bool(int(os.environ.get("BASS_KERNEL_TRACE", "0")))
    res = run_bass_kernel_spmd(nc, in_maps, core_ids=list(range(NCORES)), trace=trace)
    _CACHE["last_results"] = res

    out = np.empty((B, N, D), dtype=np.float32)
    for c in range(NCORES):
        b, n0 = c // 2, (c % 2) * NS
        out[b, n0 : n0 + NS, :] = res.results[c]["out"]
    return out


# revision 32
# speedup vs baseline: 1.3393x; 1.0058x over previous
"""MultiHeadGraphAttention TRN2 kernel, v2.

Data-parallel over (batch, query-half): core c handles batch c//2, query rows
(c%2)*1024 .. +1024.  All matmuls bf16 (fp32 PSUM); softmax + LayerNorm fp32.

v2 changes vs baseline (337us):
 - ScalarE is the wall (~130us of exp).  Everything else is arranged to hide
   under it: PSUM->SBUF projection copies moved to DVE, LayerNorm rstd uses
   ln+exp (both in the natural_log_exp_and_others table set -> no table
   thrash; Sqrt previously forced 10 table reloads mid-kernel and stalled the
   exp stream).
 - Score matmuls of a head PAIR run concurrently on disjoint PE row halves
   (K=64 each; tile_position auto-derived from base partitions 0/64).
 - Attention inner loop is software-pipelined: AV matmuls of group g-1 are
   emitted after the score matmuls of group g, so the in-order PE queue never
   blocks the next score tile (and the exp stream) behind a mask-waiting AV.
 - Input DMAs are split per consumption chunk and emitted in consumption
   order; projections start as soon as their inputs land (~4us) instead of
   after all input DMA (~38us).  Remaining projections are threaded into the
   attention stream as PE filler so the PE never idles > ~1us (HAM stays at
   K=8/8).
 - softmax denominator from an appended ones-column on V (row 64 of the AV
   output); reciprocal on DVE, partition-broadcast + normalize mul on GPSIMD.
"""

import os
import sys

import numpy as np

try:
    import concourse  # noqa: F401
except ImportError:  # harness runs from a bare dir; the repo is a fixed path
    sys.path.insert(0, "/opt/trn_rl_repo")

import ml_dtypes

B, N, M, D, H, HD = 4, 2048, 2048, 512, 8, 64
NS = 1024          # query rows per core
NCORES = 8
LN_EPS = 1e-5
BF16 = ml_dtypes.bfloat16

_CACHE = {}

# fallback knobs (read once at build)
# NOTE: reciprocal_approx_fast passes CoreSim but returns garbage on HW.
# NOTE: GPSIMD cannot access PSUM (BIR verifier) -> PSUM-reading ops on DVE.
K_XT = int(os.environ.get("K_XT", "0"))   # x_t add on gpsimd vs vector


def _build(ln_affine=True):
    import concourse.bass as bass  # noqa: F401
    import concourse.tile as tile
    from concourse import bacc, mybir
    from concourse.masks import make_identity

    f32 = mybir.dt.float32
    bf16 = mybir.dt.bfloat16
    Exp = mybir.ActivationFunctionType.Exp
    Sqrt = mybir.ActivationFunctionType.Sqrt
    sub = mybir.AluOpType.subtract
    mult = mybir.AluOpType.mult

    nc = bacc.Bacc(None, target_bir_lowering=False, debug=False)

    xqT_d = nc.dram_tensor("xqT", [D, NS], bf16, kind="ExternalInput")
    xkT_d = nc.dram_tensor("xkT", [D, M], bf16, kind="ExternalInput")
    xvT_d = nc.dram_tensor("xvT", [D, M], bf16, kind="ExternalInput")
    maskP_d = nc.dram_tensor("maskP", [2 * 8 * 128, 1024], bf16, kind="ExternalInput")
    qres_d = nc.dram_tensor("qres", [NS, D], f32, kind="ExternalInput")
    wqT_d = nc.dram_tensor("wqT", [D, D], bf16, kind="ExternalInput")
    wkT_d = nc.dram_tensor("wkT", [D, D], bf16, kind="ExternalInput")
    wvT_d = nc.dram_tensor("wvT", [D, D], bf16, kind="ExternalInput")
    woT_d = nc.dram_tensor("woT", [D, D], bf16, kind="ExternalInput")
    gamma_d = nc.dram_tensor("gamma", [1, D], f32, kind="ExternalInput")
    beta_d = nc.dram_tensor("beta", [1, D], f32, kind="ExternalInput")
    out_d = nc.dram_tensor("out", [NS, D], f32, kind="ExternalOutput")

    KC = D // 128      # 4 contraction chunks of 128
    NCH = NS // 512    # 2 query-column chunks
    MT = M // 128      # 16 key-position tiles
    MCH = M // 512     # 4 key chunks of 512
    MG = MT // 2       # 8 score groups (2 key tiles per group)
    HW = HD + 1        # per-head V slot width (64 V cols + ones col)

    with tile.TileContext(nc) as tc:
        with (
            tc.tile_pool(name="big", bufs=1) as big,
            tc.tile_pool(name="wpool", bufs=1) as wpool,
            tc.tile_pool(name="ppool", bufs=4) as ppool,
            tc.tile_pool(name="xpool", bufs=5) as xpool,
            tc.tile_pool(name="mvpool", bufs=6) as mvpool,
            tc.tile_pool(name="ypool", bufs=3) as ypool,
            tc.tile_pool(name="rpool", bufs=2) as rpool,
            tc.tile_pool(name="small", bufs=6) as small,
            tc.tile_pool(name="ps_mm", bufs=2, space="PSUM") as ps_mm,
            tc.tile_pool(name="ps_sA", bufs=1, space="PSUM") as ps_sA,
            tc.tile_pool(name="ps_sB", bufs=1, space="PSUM") as ps_sB,
            tc.tile_pool(name="ps_o", bufs=1, space="PSUM") as ps_o,
        ):
            # ---- resident SBUF tensors -----------------------------------
            xqT = big.tile([128, KC, NS], bf16, tag="xqT")
            xkT = big.tile([128, KC, M], bf16, tag="xkT")
            xvT = big.tile([128, KC, M], bf16, tag="xvT")
            maskS = big.tile([128, NCH, MG, 1024], bf16, tag="maskS")
            qT = big.tile([128, KC, NS], bf16, tag="qT")
            kT = big.tile([128, KC, M], bf16, tag="kT")
            vS = big.tile([128, MT, H * HW], bf16, tag="vS")
            oT = big.tile([128, KC, NS], bf16, tag="oT")
            wq = wpool.tile([128, KC, D], bf16, tag="wq")
            wk = wpool.tile([128, KC, D], bf16, tag="wk")
            wv = wpool.tile([128, KC, D], bf16, tag="wv")
            wo = wpool.tile([128, KC, D], bf16, tag="wo")
            gamma_b = wpool.tile([128, D], f32, tag="gamma_b")
            beta_b = wpool.tile([128, D], f32, tag="beta_b")
            gamma_1 = wpool.tile([1, D], f32, tag="gamma_1")
            beta_1 = wpool.tile([1, D], f32, tag="beta_1")
            eps_t = wpool.tile([128, 1], f32, tag="eps")
            ident = wpool.tile([128, 128], f32, tag="ident")
            make_identity(nc, ident)

            # ---- setup (no DMA dependencies; engines idle early) ---------
            nc.vector.memset(eps_t, LN_EPS)
            # ones column per head in the augmented V (softmax denominator
            # lands as row 64 of the AV matmul output)
            nc.vector.memset(
                vS[:].rearrange("p j (h x) -> p j h x", x=HW)[:, :, :, HD : HD + 1],
                1.0,
            )

            # ---- input DMAs, split per consumption chunk, priority order -
            xq_r = xqT_d[:].rearrange("(c p) n -> p c n", p=128)
            xk_r = xkT_d[:].rearrange("(c p) n -> p c n", p=128)
            xv_r = xvT_d[:].rearrange("(c p) n -> p c n", p=128)
            mk_r = maskP_d[:].rearrange("(c g p) n -> p c g n", c=NCH, g=MG)

            nc.sync.dma_start(out=wq, in_=wqT_d[:].rearrange("(c p) o -> p c o", p=128))
            for ncc in range(NCH):
                sl = slice(ncc * 512, (ncc + 1) * 512)
                nc.sync.dma_start(out=xqT[:, :, sl], in_=xq_r[:, :, sl])
            nc.sync.dma_start(out=wk, in_=wkT_d[:].rearrange("(c p) o -> p c o", p=128))
            for mc in range(MCH):
                sl = slice(mc * 512, (mc + 1) * 512)
                nc.sync.dma_start(out=xkT[:, :, sl], in_=xk_r[:, :, sl])
            nc.sync.dma_start(out=maskS[:, 0, 0, :], in_=mk_r[:, 0, 0, :])
            nc.sync.dma_start(out=maskS[:, 0, 1, :], in_=mk_r[:, 0, 1, :])
            nc.sync.dma_start(out=wv, in_=wvT_d[:].rearrange("(c p) o -> p c o", p=128))
            for jc in range(4):
                sl = slice(jc * 256, (jc + 1) * 256)
                nc.sync.dma_start(out=xvT[:, :, sl], in_=xv_r[:, :, sl])
            nc.sync.dma_start(out=maskS[:, 0, 2, :], in_=mk_r[:, 0, 2, :])
            nc.sync.dma_start(out=maskS[:, 0, 3, :], in_=mk_r[:, 0, 3, :])
            for jc in range(4, 8):
                sl = slice(jc * 256, (jc + 1) * 256)
                nc.sync.dma_start(out=xvT[:, :, sl], in_=xv_r[:, :, sl])
            for g in range(4, MG):
                nc.sync.dma_start(out=maskS[:, 0, g, :], in_=mk_r[:, 0, g, :])
            nc.sync.dma_start(out=wo, in_=woT_d[:].rearrange("(c p) o -> p c o", p=128))
            for g in range(MG):
                nc.sync.dma_start(out=maskS[:, 1, g, :], in_=mk_r[:, 1, g, :])
            nc.sync.dma_start(out=gamma_1, in_=gamma_d[:])
            nc.sync.dma_start(out=beta_1, in_=beta_d[:])
            nc.gpsimd.partition_broadcast(gamma_b, gamma_1, channels=128)
            nc.gpsimd.partition_broadcast(beta_b, beta_1, channels=128)

            # ---- projection emitters (PSUM->SBUF copies on DVE) ----------
            def q_proj(t, ncc):
                ps = ps_mm.tile([128, 512], f32, tag="mm")
                sl = slice(ncc * 512, (ncc + 1) * 512)
                for kc in range(KC):
                    nc.tensor.matmul(
                        ps,
                        lhsT=wq[:, kc, t * 128 : (t + 1) * 128],
                        rhs=xqT[:, kc, sl],
                        start=(kc == 0),
                        stop=(kc == KC - 1),
                    )
                nc.vector.tensor_copy(out=qT[:, t, sl], in_=ps)

            def k_proj(t, mc):
                ps = ps_mm.tile([128, 512], f32, tag="mm")
                sl = slice(mc * 512, (mc + 1) * 512)
                for kc in range(KC):
                    nc.tensor.matmul(
                        ps,
                        lhsT=wk[:, kc, t * 128 : (t + 1) * 128],
                        rhs=xkT[:, kc, sl],
                        start=(kc == 0),
                        stop=(kc == KC - 1),
                    )
                nc.vector.tensor_copy(out=kT[:, t, sl], in_=ps)

            def v_proj(j):
                # V[m, o] straight, scattered into per-head 65-wide slots
                ps = ps_mm.tile([128, 512], f32, tag="mm")
                for kc in range(KC):
                    nc.tensor.matmul(
                        ps,
                        lhsT=xvT[:, kc, j * 128 : (j + 1) * 128],
                        rhs=wv[:, kc, :],
                        start=(kc == 0),
                        stop=(kc == KC - 1),
                    )
                nc.vector.tensor_copy(
                    out=vS[:, j, :].rearrange("p (h x) -> p h x", x=HW)[:, :, 0:HD],
                    in_=ps[:].rearrange("p (h x) -> p h x", x=HD),
                )

            # ---- attention: head pair 2t/2t+1, software-pipelined --------
            # GPSIMD ucode note: partition_broadcast and tensor ops live in
            # DIFFERENT gpsimd libraries; alternating them costs a ~5us
            # UNLOAD_LIB/LOAD_LIB pair each time.  GPSIMD therefore runs
            # ONLY partition_broadcast; every tensor op goes to DVE.
            def normalize(po_t, h, t, nsl):
                # reciprocal via the PE-transpose dance — DVE reciprocal is
                # ~6 cycles/elem along the FREE dim, so [128,4] (0.2us)
                # beats [1,512] (3us)
                po2 = (h % 2) * 64
                dS = rpool.tile([1, 512], f32, tag="dS")
                nc.vector.tensor_copy(out=dS, in_=po_t[HD : HD + 1, :])
                scr = ps_mm.tile([128, 512], f32, tag="mm")
                dT = scr[:, 0:4]
                rrow = scr[0:1, 0:512]
                for c in range(KC):
                    nc.tensor.transpose(
                        dT[:, c : c + 1], dS[:, c * 128 : (c + 1) * 128],
                        ident[0:1, 0:1],
                    )
                rT = small.tile([128, 4], f32, tag="rT")
                nc.vector.reciprocal(rT, dT)
                for c in range(KC):
                    nc.tensor.transpose(
                        rrow[:, c * 128 : (c + 1) * 128], rT[:, c : c + 1], ident
                    )
                recip_s = rpool.tile([1, 512], f32, tag="recip")
                nc.vector.tensor_copy(out=recip_s, in_=rrow)
                rb = rpool.tile([64, 512], f32, tag="rb")
                nc.gpsimd.partition_broadcast(rb, recip_s, channels=64)
                nc.vector.tensor_mul(oT[po2 : po2 + 64, t, nsl], po_t[0:HD, :], rb)

            # one continuous stream over all (pair, chunk, group) units; the
            # AV matmuls run one unit behind the score/exp/mask front so the
            # in-order PE queue never blocks the exp stream, and the pipeline
            # never drains at pair boundaries
            pend = {}   # live pair state: (t, ncc) -> (poE, poO, nsl)
            pts = {}    # unit -> (ptA, ptB)

            def emit_av(unit):
                t, ncc, g = unit
                poE, poO, _ = pend[(t, ncc)]
                slotE = slice((2 * t) * HW, (2 * t + 1) * HW)
                slotO = slice((2 * t + 1) * HW, (2 * t + 2) * HW)
                ptA, ptB = pts.pop(unit)
                for u in range(2):
                    j = 2 * g + u
                    usl = slice(u * 512, (u + 1) * 512)
                    nc.tensor.matmul(
                        poE, lhsT=vS[:, j, slotE], rhs=ptA[:, usl],
                        start=(j == 0), stop=(j == MT - 1),
                    )
                    nc.tensor.matmul(
                        poO, lhsT=vS[:, j, slotO], rhs=ptB[:, usl],
                        start=(j == 0), stop=(j == MT - 1),
                    )
                if g == MG - 1:
                    poE, poO, nsl = pend.pop((t, ncc))
                    normalize(poE, 2 * t, t, nsl)
                    normalize(poO, 2 * t + 1, t, nsl)

            def attend_all(pair_order, fillmap):
                units = [(t, ncc, g) for (t, ncc) in pair_order for g in range(MG)]
                prev = None
                for unit in units:
                    t, ncc, g = unit
                    nsl = slice(ncc * 512, (ncc + 1) * 512)
                    if g == 0:
                        poE_new = ps_o.tile([HW, 512], f32, tag="poE")
                        poO_new = ps_o.tile([HW, 512], f32, tag="poO")
                        pend[(t, ncc)] = (poE_new, poO_new, nsl)
                    poE, poO, _ = pend[(t, ncc)]
                    psA = ps_sA.tile([128, 1024], f32, tag="sA")
                    psB = ps_sB.tile([128, 1024], f32, tag="sB")
                    for u in range(2):
                        j = 2 * g + u
                        usl = slice(u * 512, (u + 1) * 512)
                        # two heads on disjoint PE row halves -> concurrent
                        nc.tensor.matmul(
                            psA[:, usl],
                            lhsT=kT[0:64, t, j * 128 : (j + 1) * 128],
                            rhs=qT[0:64, t, nsl],
                            start=True, stop=True,
                        )
                        nc.tensor.matmul(
                            psB[:, usl],
                            lhsT=kT[64:128, t, j * 128 : (j + 1) * 128],
                            rhs=qT[64:128, t, nsl],
                            start=True, stop=True,
                        )
                    ptA = ppool.tile([128, 1024], bf16, tag="pt")
                    nc.scalar.activation(ptA, psA, Exp)
                    ptB = ppool.tile([128, 1024], bf16, tag="pt")
                    nc.scalar.activation(ptB, psB, Exp)
                    nc.vector.tensor_mul(ptA, ptA, maskS[:, ncc, g, :])
                    nc.vector.tensor_mul(ptB, ptB, maskS[:, ncc, g, :])
                    pts[unit] = (ptA, ptB)
                    for f in fillmap.get((t, ncc), {}).get(g, ()):
                        f()
                    if prev is not None:
                        emit_av(prev)
                    prev = unit
                emit_av(prev)

            # ---- output projection + residual + LayerNorm ----------------
            qres_r = qres_d[:].rearrange("(t p) d -> p t d", p=128)
            out_r = out_d[:].rearrange("(t p) d -> p t d", p=128)
            ot_state = {}

            def out_front(nt):
                ps = ps_mm.tile([128, 512], f32, tag="mm")
                for a in range(KC):
                    nc.tensor.matmul(
                        ps,
                        lhsT=oT[:, a, nt * 128 : (nt + 1) * 128],
                        rhs=wo[:, a, :],
                        start=(a == 0),
                        stop=(a == KC - 1),
                    )
                qres_t = ypool.tile([128, D], f32, tag="qres")
                nc.sync.dma_start(out=qres_t, in_=qres_r[:, nt, :])
                x_t = xpool.tile([128, D], f32, tag="x")
                if K_XT:
                    nc.gpsimd.tensor_add(x_t, ps, qres_t)
                else:
                    nc.vector.tensor_add(x_t, ps, qres_t)
                stats = small.tile([128, 6], f32, tag="stats")
                nc.vector.bn_stats(out=stats, in_=x_t)
                mv = mvpool.tile([128, 2], f32, tag="mv")
                nc.vector.bn_aggr(out=mv, in_=stats)
                ot_state[nt] = (x_t, mv)

            rstd_store = {}

            def rstd_batch(nts):
                # one Sqrt activation for a wave of tiles -> 2 ACT table
                # switches per wave instead of 2 per tile
                vcol = small.tile([128, 4], f32, tag="vcol")
                for i, nt in enumerate(nts):
                    nc.vector.tensor_copy(out=vcol[:, i : i + 1],
                                          in_=ot_state[nt][1][:, 1:2])
                sd = small.tile([128, 4], f32, tag="sd")
                nc.scalar.activation(sd, vcol, Sqrt, bias=eps_t)
                rs = mvpool.tile([128, 4], f32, tag="rs")
                nc.vector.reciprocal(rs, sd)
                for i, nt in enumerate(nts):
                    rstd_store[nt] = (rs, i)

            def out_back(nt, tail=False):
                x_t, mv = ot_state.pop(nt)
                rs, i = rstd_store.pop(nt)
                xn = ypool.tile([128, D], f32, tag="xn")
                nc.vector.tensor_scalar(
                    out=xn, in0=x_t, scalar1=mv[:, 0:1], scalar2=rs[:, i : i + 1],
                    op0=sub, op1=mult,
                )
                if ln_affine:  # on DVE: gpsimd is reserved for broadcasts
                    y_t = ypool.tile([128, D], f32, tag="y")
                    nc.vector.tensor_mul(y_t, xn, gamma_b)
                    nc.vector.tensor_add(y_t, y_t, beta_b)
                else:          # gamma==1, beta==0 (checked host-side)
                    y_t = xn
                nc.sync.dma_start(out=out_r[:, nt, :], in_=y_t)

            # ---- emission schedule ---------------------------------------
            # PE pre-heat: the HAM clock gate needs ~3.4us of sustained
            # matmul activity to unthrottle 1.2->2.4 GHz, and re-throttles
            # after a ~3.4us idle window.  The PE would otherwise sit idle
            # for ~14us of input DMA; a dummy burst here latches K=8/8 right
            # before the projection stream begins, and the steady-state
            # pipeline's gaps (<1.5us) keep it latched.
            for w in range(128):
                ps = ps_mm.tile([128, 512], f32, tag="mm")
                nc.tensor.matmul(ps[:, 0:128], lhsT=ident[:, 0:128],
                                 rhs=ident[:, 0:128], start=True, stop=True)

            # ramp: just enough projection work for pair 0 + first AV tiles
            q_proj(0, 0)
            q_proj(0, 1)
            for mc in range(MCH):
                k_proj(0, mc)
            v_proj(0)
            v_proj(1)

            def C(f, *a):
                return lambda: f(*a)

            # pair-0 fillers: V tiles JIT (AV of group g needs v(2g,2g+1);
            # slot g supplies v(2g+2,2g+3)); pair-p prereqs (qT/kT complete)
            # must be emitted before pair p starts
            f00 = {
                0: (C(v_proj, 2), C(v_proj, 3)),
                1: (C(v_proj, 4), C(v_proj, 5)),
                2: (C(v_proj, 6), C(v_proj, 7)),
                3: (C(v_proj, 8), C(v_proj, 9)),
                4: (C(v_proj, 10), C(v_proj, 11)),
                5: (C(v_proj, 12), C(v_proj, 13)),
                6: (C(v_proj, 14), C(v_proj, 15), C(q_proj, 1, 0)),
                7: (C(q_proj, 1, 1), C(k_proj, 1, 0)),
            }
            # k(t,mc) feeds score groups 2mc..2mc+1 of pair t: later chunks
            # can trail into pair t itself as long as they stay 2 groups ahead
            f10 = {
                0: (C(k_proj, 1, 1),),
                1: (C(k_proj, 1, 2), C(k_proj, 1, 3)),
                3: (C(q_proj, 2, 0),),
                4: (C(q_proj, 2, 1),),
                5: (C(k_proj, 2, 0),),
                6: (C(k_proj, 2, 1),),
                7: (C(k_proj, 2, 2), C(k_proj, 2, 3)),
            }
            f20 = {
                0: (C(q_proj, 3, 0),),
                1: (C(q_proj, 3, 1),),
                4: (C(k_proj, 3, 0),),
                5: (C(k_proj, 3, 1),),
                6: (C(k_proj, 3, 2), C(k_proj, 3, 3)),
            }
            # Scalar queue is strict FIFO: the wave-A Sqrt must enter it only
            # when its bn-stats deps are long done, else every later exp
            # stalls behind it.  fronts 0-3 early in ncc1, Sqrt a full pair
            # later, backs on the last pair.
            f01 = {2: (C(out_front, 0),), 4: (C(out_front, 1),),
                   6: (C(out_front, 2),)}
            f11 = {0: (C(out_front, 3),)}
            f21 = {4: (C(rstd_batch, (0, 1, 2, 3)),)}
            f31 = {0: (C(out_back, 0),), 2: (C(out_back, 1),),
                   4: (C(out_back, 2),), 6: (C(out_back, 3),)}

            pair_order = [(0, 0), (1, 0), (2, 0), (3, 0),
                          (0, 1), (1, 1), (2, 1), (3, 1)]
            fillmap = {(0, 0): f00, (1, 0): f10, (2, 0): f20,
                       (0, 1): f01, (1, 1): f11, (2, 1): f21, (3, 1): f31}
            attend_all(pair_order, fillmap)
            out_front(4)
            out_front(5)
            out_front(6)
            out_front(7)
            rstd_batch((4, 5, 6, 7))
            out_back(4, tail=True)
            out_back(5, tail=True)
            out_back(6, tail=True)
            out_back(7, tail=True)

    nc.compile()
    return nc


def kernel(**inputs):
    from concourse.bass_utils import run_bass_kernel_spmd

    gamma_a = np.asarray(inputs["gamma"], dtype=np.float32)
    beta_a = np.asarray(inputs["beta"], dtype=np.float32)
    ln_affine = bool(np.any(gamma_a != 1.0) or np.any(beta_a != 0.0))
    ck = ("nc", ln_affine)
    if ck not in _CACHE:
        _CACHE[ck] = _build(ln_affine)
    nc = _CACHE[ck]

    query = np.asarray(inputs["query"], dtype=np.float32)
    key = np.asarray(inputs["key"], dtype=np.float32)
    value = np.asarray(inputs["value"], dtype=np.float32)
    mask = np.asarray(inputs["mask"])
    WQ = np.asarray(inputs["WQ"], dtype=np.float32)
    WK = np.asarray(inputs["WK"], dtype=np.float32)
    WV = np.asarray(inputs["WV"], dtype=np.float32)
    WO = np.asarray(inputs["WO"], dtype=np.float32)
    bO = np.asarray(inputs["bO"], dtype=np.float32)
    gamma = np.asarray(inputs["gamma"], dtype=np.float32)
    beta = np.asarray(inputs["beta"], dtype=np.float32)

    scale = np.float32(1.0 / np.sqrt(HD))
    wqT = np.ascontiguousarray(WQ.T * scale).astype(BF16)
    wkT = np.ascontiguousarray(WK.T).astype(BF16)
    wvT = np.ascontiguousarray(WV.T).astype(BF16)
    woT = np.ascontiguousarray(WO.T).astype(BF16)
    gamma_in = gamma.reshape(1, D)
    beta_in = beta.reshape(1, D)
    mask_bin = (mask != 0)

    in_maps = []
    for c in range(NCORES):
        b, n0 = c // 2, (c % 2) * NS
        # mask, transposed and prepacked per (n-chunk, score-group):
        # maskP[ncc, g, p, u*512+nn] = maskT[g*256+u*128+p, ncc*512+nn]
        mT = np.ascontiguousarray(mask_bin[b, n0 : n0 + NS, :].T)  # [M, NS]
        mP = (
            mT.reshape(8, 2, 128, 2, 512)
            .transpose(3, 0, 2, 1, 4)
            .reshape(2 * 8 * 128, 1024)
        )
        in_maps.append({
            "xqT": np.ascontiguousarray(query[b, n0 : n0 + NS, :].T).astype(BF16),
            "xkT": np.ascontiguousarray(key[b].T).astype(BF16),
            "xvT": np.ascontiguousarray(value[b].T).astype(BF16),
            "maskP": np.ascontiguousarray(mP).astype(BF16),
            "qres": np.ascontiguousarray(query[b, n0 : n0 + NS, :] + bO[None, :]),
            "wqT": wqT, "wkT": wkT, "wvT": wvT, "woT": woT,
            "gamma": gamma_in, "beta": beta_in,
        })

    trace = bool(int(os.environ.get("BASS_KERNEL_TRACE", "0")))
    res = run_bass_kernel_spmd(nc, in_maps, core_ids=list(range(NCORES)), trace=trace)
    _CACHE["last_results"] = res

    out = np.empty((B, N, D), dtype=np.float32)
    for c in range(NCORES):
        b, n0 = c // 2, (c % 2) * NS
        out[b, n0 : n0 + NS, :] = res.results[c]["out"]
    return out


# revision 34
# speedup vs baseline: 1.3693x; 1.0224x over previous
"""MultiHeadGraphAttention TRN2 kernel, v2.

Data-parallel over (batch, query-half): core c handles batch c//2, query rows
(c%2)*1024 .. +1024.  All matmuls bf16 (fp32 PSUM); softmax + LayerNorm fp32.

v2 changes vs baseline (337us):
 - ScalarE is the wall (~130us of exp).  Everything else is arranged to hide
   under it: PSUM->SBUF projection copies moved to DVE, LayerNorm rstd uses
   ln+exp (both in the natural_log_exp_and_others table set -> no table
   thrash; Sqrt previously forced 10 table reloads mid-kernel and stalled the
   exp stream).
 - Score matmuls of a head PAIR run concurrently on disjoint PE row halves
   (K=64 each; tile_position auto-derived from base partitions 0/64).
 - Attention inner loop is software-pipelined: AV matmuls of group g-1 are
   emitted after the score matmuls of group g, so the in-order PE queue never
   blocks the next score tile (and the exp stream) behind a mask-waiting AV.
 - Input DMAs are split per consumption chunk and emitted in consumption
   order; projections start as soon as their inputs land (~4us) instead of
   after all input DMA (~38us).  Remaining projections are threaded into the
   attention stream as PE filler so the PE never idles > ~1us (HAM stays at
   K=8/8).
 - softmax denominator from an appended ones-column on V (row 64 of the AV
   output); reciprocal on DVE, partition-broadcast + normalize mul on GPSIMD.
"""

import os
import sys

import numpy as np

try:
    import concourse  # noqa: F401
except ImportError:  # harness runs from a bare dir; the repo is a fixed path
    sys.path.insert(0, "/opt/trn_rl_repo")

import ml_dtypes

B, N, M, D, H, HD = 4, 2048, 2048, 512, 8, 64
NS = 1024          # query rows per core
NCORES = 8
LN_EPS = 1e-5
BF16 = ml_dtypes.bfloat16

_CACHE = {}

# fallback knobs (read once at build)
# NOTE: reciprocal_approx_fast passes CoreSim but returns garbage on HW.
# NOTE: GPSIMD cannot access PSUM (BIR verifier) -> PSUM-reading ops on DVE.
K_XT = int(os.environ.get("K_XT", "0"))   # x_t add on gpsimd vs vector


def _build(ln_affine=True):
    import concourse.bass as bass  # noqa: F401
    import concourse.tile as tile
    from concourse import bacc, mybir
    from concourse.masks import make_identity

    f32 = mybir.dt.float32
    bf16 = mybir.dt.bfloat16
    Exp = mybir.ActivationFunctionType.Exp
    Sqrt = mybir.ActivationFunctionType.Sqrt
    sub = mybir.AluOpType.subtract
    mult = mybir.AluOpType.mult

    nc = bacc.Bacc(None, target_bir_lowering=False, debug=False)

    xqT_d = nc.dram_tensor("xqT", [D, NS], bf16, kind="ExternalInput")
    xkT_d = nc.dram_tensor("xkT", [D, M], bf16, kind="ExternalInput")
    xvT_d = nc.dram_tensor("xvT", [D, M], bf16, kind="ExternalInput")
    maskP_d = nc.dram_tensor("maskP", [2 * 8 * 128, 1024], bf16, kind="ExternalInput")
    qres_d = nc.dram_tensor("qres", [NS, D], f32, kind="ExternalInput")
    wqT_d = nc.dram_tensor("wqT", [D, D], bf16, kind="ExternalInput")
    wkT_d = nc.dram_tensor("wkT", [D, D], bf16, kind="ExternalInput")
    wvT_d = nc.dram_tensor("wvT", [D, D], bf16, kind="ExternalInput")
    woT_d = nc.dram_tensor("woT", [D, D], bf16, kind="ExternalInput")
    gamma_d = nc.dram_tensor("gamma", [1, D], f32, kind="ExternalInput")
    beta_d = nc.dram_tensor("beta", [1, D], f32, kind="ExternalInput")
    out_d = nc.dram_tensor("out", [NS, D], f32, kind="ExternalOutput")

    KC = D // 128      # 4 contraction chunks of 128
    NCH = NS // 512    # 2 query-column chunks
    MT = M // 128      # 16 key-position tiles
    MCH = M // 512     # 4 key chunks of 512
    MG = MT // 2       # 8 score groups (2 key tiles per group)
    HW = HD + 1        # per-head V slot width (64 V cols + ones col)

    with tile.TileContext(nc) as tc:
        with (
            tc.tile_pool(name="big", bufs=1) as big,
            tc.tile_pool(name="wpool", bufs=1) as wpool,
            tc.tile_pool(name="ppool", bufs=4) as ppool,
            tc.tile_pool(name="xpool", bufs=5) as xpool,
            tc.tile_pool(name="mvpool", bufs=6) as mvpool,
            tc.tile_pool(name="ypool", bufs=3) as ypool,
            tc.tile_pool(name="rpool", bufs=2) as rpool,
            tc.tile_pool(name="small", bufs=6) as small,
            tc.tile_pool(name="ps_mm", bufs=2, space="PSUM") as ps_mm,
            tc.tile_pool(name="ps_sA", bufs=1, space="PSUM") as ps_sA,
            tc.tile_pool(name="ps_sB", bufs=1, space="PSUM") as ps_sB,
            tc.tile_pool(name="ps_o", bufs=1, space="PSUM") as ps_o,
        ):
            # ---- resident SBUF tensors -----------------------------------
            xqT = big.tile([128, KC, NS], bf16, tag="xqT")
            xkT = big.tile([128, KC, M], bf16, tag="xkT")
            xvT = big.tile([128, KC, M], bf16, tag="xvT")
            maskS = big.tile([128, NCH, MG, 1024], bf16, tag="maskS")
            qT = big.tile([128, KC, NS], bf16, tag="qT")
            kT = big.tile([128, KC, M], bf16, tag="kT")
            vS = big.tile([128, MT, H * HW], bf16, tag="vS")
            oT = big.tile([128, KC, NS], bf16, tag="oT")
            wq = wpool.tile([128, KC, D], bf16, tag="wq")
            wk = wpool.tile([128, KC, D], bf16, tag="wk")
            wv = wpool.tile([128, KC, D], bf16, tag="wv")
            wo = wpool.tile([128, KC, D], bf16, tag="wo")
            gamma_b = wpool.tile([128, D], f32, tag="gamma_b")
            beta_b = wpool.tile([128, D], f32, tag="beta_b")
            gamma_1 = wpool.tile([1, D], f32, tag="gamma_1")
            beta_1 = wpool.tile([1, D], f32, tag="beta_1")
            eps_t = wpool.tile([128, 1], f32, tag="eps")
            ident = wpool.tile([128, 128], f32, tag="ident")
            make_identity(nc, ident)

            # ---- setup (no DMA dependencies; engines idle early) ---------
            nc.vector.memset(eps_t, LN_EPS)
            # ones column per head in the augmented V (softmax denominator
            # lands as row 64 of the AV matmul output)
            nc.vector.memset(
                vS[:].rearrange("p j (h x) -> p j h x", x=HW)[:, :, :, HD : HD + 1],
                1.0,
            )

            # ---- input DMAs, split per consumption chunk, priority order -
            xq_r = xqT_d[:].rearrange("(c p) n -> p c n", p=128)
            xk_r = xkT_d[:].rearrange("(c p) n -> p c n", p=128)
            xv_r = xvT_d[:].rearrange("(c p) n -> p c n", p=128)
            mk_r = maskP_d[:].rearrange("(c g p) n -> p c g n", c=NCH, g=MG)

            nc.sync.dma_start(out=wq, in_=wqT_d[:].rearrange("(c p) o -> p c o", p=128))
            for ncc in range(NCH):
                sl = slice(ncc * 512, (ncc + 1) * 512)
                nc.sync.dma_start(out=xqT[:, :, sl], in_=xq_r[:, :, sl])
            nc.sync.dma_start(out=wk, in_=wkT_d[:].rearrange("(c p) o -> p c o", p=128))
            for mc in range(MCH):
                sl = slice(mc * 512, (mc + 1) * 512)
                nc.sync.dma_start(out=xkT[:, :, sl], in_=xk_r[:, :, sl])
            nc.sync.dma_start(out=maskS[:, 0, 0, :], in_=mk_r[:, 0, 0, :])
            nc.sync.dma_start(out=maskS[:, 0, 1, :], in_=mk_r[:, 0, 1, :])
            nc.sync.dma_start(out=wv, in_=wvT_d[:].rearrange("(c p) o -> p c o", p=128))
            for jc in range(4):
                sl = slice(jc * 256, (jc + 1) * 256)
                nc.sync.dma_start(out=xvT[:, :, sl], in_=xv_r[:, :, sl])
            nc.sync.dma_start(out=maskS[:, 0, 2, :], in_=mk_r[:, 0, 2, :])
            nc.sync.dma_start(out=maskS[:, 0, 3, :], in_=mk_r[:, 0, 3, :])
            for jc in range(4, 8):
                sl = slice(jc * 256, (jc + 1) * 256)
                nc.sync.dma_start(out=xvT[:, :, sl], in_=xv_r[:, :, sl])
            for g in range(4, MG):
                nc.sync.dma_start(out=maskS[:, 0, g, :], in_=mk_r[:, 0, g, :])
            nc.sync.dma_start(out=wo, in_=woT_d[:].rearrange("(c p) o -> p c o", p=128))
            for g in range(MG):
                nc.sync.dma_start(out=maskS[:, 1, g, :], in_=mk_r[:, 1, g, :])
            nc.sync.dma_start(out=gamma_1, in_=gamma_d[:])
            nc.sync.dma_start(out=beta_1, in_=beta_d[:])
            nc.gpsimd.partition_broadcast(gamma_b, gamma_1, channels=128)
            nc.gpsimd.partition_broadcast(beta_b, beta_1, channels=128)

            # ---- projection emitters (PSUM->SBUF copies on DVE) ----------
            def q_proj(t, ncc):
                ps = ps_mm.tile([128, 512], f32, tag="mm")
                sl = slice(ncc * 512, (ncc + 1) * 512)
                for kc in range(KC):
                    nc.tensor.matmul(
                        ps,
                        lhsT=wq[:, kc, t * 128 : (t + 1) * 128],
                        rhs=xqT[:, kc, sl],
                        start=(kc == 0),
                        stop=(kc == KC - 1),
                    )
                nc.vector.tensor_copy(out=qT[:, t, sl], in_=ps)

            def k_proj(t, mc):
                ps = ps_mm.tile([128, 512], f32, tag="mm")
                sl = slice(mc * 512, (mc + 1) * 512)
                for kc in range(KC):
                    nc.tensor.matmul(
                        ps,
                        lhsT=wk[:, kc, t * 128 : (t + 1) * 128],
                        rhs=xkT[:, kc, sl],
                        start=(kc == 0),
                        stop=(kc == KC - 1),
                    )
                nc.vector.tensor_copy(out=kT[:, t, sl], in_=ps)

            def v_proj(j):
                # V[m, o] straight, scattered into per-head 65-wide slots
                ps = ps_mm.tile([128, 512], f32, tag="mm")
                for kc in range(KC):
                    nc.tensor.matmul(
                        ps,
                        lhsT=xvT[:, kc, j * 128 : (j + 1) * 128],
                        rhs=wv[:, kc, :],
                        start=(kc == 0),
                        stop=(kc == KC - 1),
                    )
                nc.vector.tensor_copy(
                    out=vS[:, j, :].rearrange("p (h x) -> p h x", x=HW)[:, :, 0:HD],
                    in_=ps[:].rearrange("p (h x) -> p h x", x=HD),
                )

            # ---- attention: head pair 2t/2t+1, software-pipelined --------
            # GPSIMD ucode note: partition_broadcast and tensor ops live in
            # DIFFERENT gpsimd libraries; alternating them costs a ~5us
            # UNLOAD_LIB/LOAD_LIB pair each time.  GPSIMD therefore runs
            # ONLY partition_broadcast; every tensor op goes to DVE.
            def normalize_flat(po_t, h, t, nsl):
                # latency-optimized variant for the final pairs: 4 queue hops
                # instead of 7.  The 3us one-lane reciprocal is fine when the
                # only consumer is the kernel tail.
                po2 = (h % 2) * 64
                dS = rpool.tile([1, 512], f32, tag="dS")
                nc.vector.tensor_copy(out=dS, in_=po_t[HD : HD + 1, :])
                recip_s = rpool.tile([1, 512], f32, tag="recip")
                nc.vector.reciprocal(recip_s, dS)
                rb = rpool.tile([64, 512], f32, tag="rb")
                nc.gpsimd.partition_broadcast(rb, recip_s, channels=64)
                nc.vector.tensor_mul(oT[po2 : po2 + 64, t, nsl], po_t[0:HD, :], rb)

            def normalize(po_t, h, t, nsl):
                # reciprocal via the PE-transpose dance — DVE reciprocal is
                # ~6 cycles/elem along the FREE dim, so [128,4] (0.2us)
                # beats [1,512] (3us)
                po2 = (h % 2) * 64
                dS = rpool.tile([1, 512], f32, tag="dS")
                nc.vector.tensor_copy(out=dS, in_=po_t[HD : HD + 1, :])
                scr = ps_mm.tile([128, 512], f32, tag="mm")
                dT = scr[:, 0:4]
                rrow = scr[0:1, 0:512]
                for c in range(KC):
                    nc.tensor.transpose(
                        dT[:, c : c + 1], dS[:, c * 128 : (c + 1) * 128],
                        ident[0:1, 0:1],
                    )
                rT = small.tile([128, 4], f32, tag="rT")
                nc.vector.reciprocal(rT, dT)
                for c in range(KC):
                    nc.tensor.transpose(
                        rrow[:, c * 128 : (c + 1) * 128], rT[:, c : c + 1], ident
                    )
                recip_s = rpool.tile([1, 512], f32, tag="recip")
                nc.vector.tensor_copy(out=recip_s, in_=rrow)
                rb = rpool.tile([64, 512], f32, tag="rb")
                nc.gpsimd.partition_broadcast(rb, recip_s, channels=64)
                nc.vector.tensor_mul(oT[po2 : po2 + 64, t, nsl], po_t[0:HD, :], rb)

            # one continuous stream over all (pair, chunk, group) units; the
            # AV matmuls run one unit behind the score/exp/mask front so the
            # in-order PE queue never blocks the exp stream, and the pipeline
            # never drains at pair boundaries
            pend = {}   # live pair state: (t, ncc) -> (poE, poO, nsl)
            pts = {}    # unit -> (ptA, ptB)

            def emit_av(unit):
                t, ncc, g = unit
                poE, poO, _ = pend[(t, ncc)]
                slotE = slice((2 * t) * HW, (2 * t + 1) * HW)
                slotO = slice((2 * t + 1) * HW, (2 * t + 2) * HW)
                ptA, ptB = pts.pop(unit)
                for u in range(2):
                    j = 2 * g + u
                    usl = slice(u * 512, (u + 1) * 512)
                    nc.tensor.matmul(
                        poE, lhsT=vS[:, j, slotE], rhs=ptA[:, usl],
                        start=(j == 0), stop=(j == MT - 1),
                    )
                    nc.tensor.matmul(
                        poO, lhsT=vS[:, j, slotO], rhs=ptB[:, usl],
                        start=(j == 0), stop=(j == MT - 1),
                    )
                if g == MG - 1:
                    poE, poO, nsl = pend.pop((t, ncc))
                    norm = normalize_flat if (ncc == 1 and t >= 2) else normalize
                    norm(poE, 2 * t, t, nsl)
                    norm(poO, 2 * t + 1, t, nsl)

            def attend_all(pair_order, fillmap):
                units = [(t, ncc, g) for (t, ncc) in pair_order for g in range(MG)]
                prev = None
                for unit in units:
                    t, ncc, g = unit
                    nsl = slice(ncc * 512, (ncc + 1) * 512)
                    if g == 0:
                        poE_new = ps_o.tile([HW, 512], f32, tag="poE")
                        poO_new = ps_o.tile([HW, 512], f32, tag="poO")
                        pend[(t, ncc)] = (poE_new, poO_new, nsl)
                    poE, poO, _ = pend[(t, ncc)]
                    psA = ps_sA.tile([128, 1024], f32, tag="sA")
                    psB = ps_sB.tile([128, 1024], f32, tag="sB")
                    for u in range(2):
                        j = 2 * g + u
                        usl = slice(u * 512, (u + 1) * 512)
                        # two heads on disjoint PE row halves -> concurrent
                        nc.tensor.matmul(
                            psA[:, usl],
                            lhsT=kT[0:64, t, j * 128 : (j + 1) * 128],
                            rhs=qT[0:64, t, nsl],
                            start=True, stop=True,
                        )
                        nc.tensor.matmul(
                            psB[:, usl],
                            lhsT=kT[64:128, t, j * 128 : (j + 1) * 128],
                            rhs=qT[64:128, t, nsl],
                            start=True, stop=True,
                        )
                    ptA = ppool.tile([128, 1024], bf16, tag="pt")
                    nc.scalar.activation(ptA, psA, Exp)
                    ptB = ppool.tile([128, 1024], bf16, tag="pt")
                    nc.scalar.activation(ptB, psB, Exp)
                    nc.vector.tensor_mul(ptA, ptA, maskS[:, ncc, g, :])
                    nc.vector.tensor_mul(ptB, ptB, maskS[:, ncc, g, :])
                    pts[unit] = (ptA, ptB)
                    for f in fillmap.get((t, ncc), {}).get(g, ()):
                        f()
                    if prev is not None:
                        emit_av(prev)
                    prev = unit
                emit_av(prev)

            # ---- output projection + residual + LayerNorm ----------------
            qres_r = qres_d[:].rearrange("(t p) d -> p t d", p=128)
            out_r = out_d[:].rearrange("(t p) d -> p t d", p=128)
            ot_state = {}

            def out_front(nt):
                ps = ps_mm.tile([128, 512], f32, tag="mm")
                for a in range(KC):
                    nc.tensor.matmul(
                        ps,
                        lhsT=oT[:, a, nt * 128 : (nt + 1) * 128],
                        rhs=wo[:, a, :],
                        start=(a == 0),
                        stop=(a == KC - 1),
                    )
                qres_t = ypool.tile([128, D], f32, tag="qres")
                nc.sync.dma_start(out=qres_t, in_=qres_r[:, nt, :])
                x_t = xpool.tile([128, D], f32, tag="x")
                if K_XT:
                    nc.gpsimd.tensor_add(x_t, ps, qres_t)
                else:
                    nc.vector.tensor_add(x_t, ps, qres_t)
                stats = small.tile([128, 6], f32, tag="stats")
                nc.vector.bn_stats(out=stats, in_=x_t)
                mv = mvpool.tile([128, 2], f32, tag="mv")
                nc.vector.bn_aggr(out=mv, in_=stats)
                ot_state[nt] = (x_t, mv)

            rstd_store = {}

            def rstd_batch(nts):
                # one Sqrt activation for a wave of tiles -> 2 ACT table
                # switches per wave instead of 2 per tile
                vcol = small.tile([128, 4], f32, tag="vcol")
                for i, nt in enumerate(nts):
                    nc.vector.tensor_copy(out=vcol[:, i : i + 1],
                                          in_=ot_state[nt][1][:, 1:2])
                sd = small.tile([128, 4], f32, tag="sd")
                nc.scalar.activation(sd, vcol, Sqrt, bias=eps_t)
                rs = mvpool.tile([128, 4], f32, tag="rs")
                nc.vector.reciprocal(rs, sd)
                for i, nt in enumerate(nts):
                    rstd_store[nt] = (rs, i)

            def out_back(nt, tail=False):
                x_t, mv = ot_state.pop(nt)
                rs, i = rstd_store.pop(nt)
                xn = ypool.tile([128, D], f32, tag="xn")
                nc.vector.tensor_scalar(
                    out=xn, in0=x_t, scalar1=mv[:, 0:1], scalar2=rs[:, i : i + 1],
                    op0=sub, op1=mult,
                )
                if ln_affine:  # on DVE: gpsimd is reserved for broadcasts
                    y_t = ypool.tile([128, D], f32, tag="y")
                    nc.vector.tensor_mul(y_t, xn, gamma_b)
                    nc.vector.tensor_add(y_t, y_t, beta_b)
                else:          # gamma==1, beta==0 (checked host-side)
                    y_t = xn
                nc.sync.dma_start(out=out_r[:, nt, :], in_=y_t)

            # ---- emission schedule ---------------------------------------
            # PE pre-heat: the HAM clock gate needs ~3.4us of sustained
            # matmul activity to unthrottle 1.2->2.4 GHz, and re-throttles
            # after a ~3.4us idle window.  The PE would otherwise sit idle
            # for ~14us of input DMA; a dummy burst here latches K=8/8 right
            # before the projection stream begins, and the steady-state
            # pipeline's gaps (<1.5us) keep it latched.
            for w in range(128):
                ps = ps_mm.tile([128, 512], f32, tag="mm")
                nc.tensor.matmul(ps[:, 0:128], lhsT=ident[:, 0:128],
                                 rhs=ident[:, 0:128], start=True, stop=True)

            # ramp: just enough projection work for pair 0 + first AV tiles
            q_proj(0, 0)
            q_proj(0, 1)
            for mc in range(MCH):
                k_proj(0, mc)
            v_proj(0)
            v_proj(1)

            def C(f, *a):
                return lambda: f(*a)

            # pair-0 fillers: V tiles JIT (AV of group g needs v(2g,2g+1);
            # slot g supplies v(2g+2,2g+3)); pair-p prereqs (qT/kT complete)
            # must be emitted before pair p starts
            f00 = {
                0: (C(v_proj, 2), C(v_proj, 3)),
                1: (C(v_proj, 4), C(v_proj, 5)),
                2: (C(v_proj, 6), C(v_proj, 7)),
                3: (C(v_proj, 8), C(v_proj, 9)),
                4: (C(v_proj, 10), C(v_proj, 11)),
                5: (C(v_proj, 12), C(v_proj, 13)),
                6: (C(v_proj, 14), C(v_proj, 15), C(q_proj, 1, 0)),
                7: (C(q_proj, 1, 1), C(k_proj, 1, 0)),
            }
            # k(t,mc) feeds score groups 2mc..2mc+1 of pair t: later chunks
            # can trail into pair t itself as long as they stay 2 groups ahead
            f10 = {
                0: (C(k_proj, 1, 1),),
                1: (C(k_proj, 1, 2), C(k_proj, 1, 3)),
                3: (C(q_proj, 2, 0),),
                4: (C(q_proj, 2, 1),),
                5: (C(k_proj, 2, 0),),
                6: (C(k_proj, 2, 1),),
                7: (C(k_proj, 2, 2), C(k_proj, 2, 3)),
            }
            f20 = {
                0: (C(q_proj, 3, 0),),
                1: (C(q_proj, 3, 1),),
                4: (C(k_proj, 3, 0),),
                5: (C(k_proj, 3, 1),),
                6: (C(k_proj, 3, 2), C(k_proj, 3, 3)),
            }
            # Scalar queue is strict FIFO: the wave-A Sqrt must enter it only
            # when its bn-stats deps are long done, else every later exp
            # stalls behind it.  fronts 0-3 early in ncc1, Sqrt a full pair
            # later, backs on the last pair.
            f01 = {2: (C(out_front, 0),), 4: (C(out_front, 1),),
                   6: (C(out_front, 2),)}
            f11 = {0: (C(out_front, 3),)}
            f21 = {4: (C(rstd_batch, (0, 1, 2, 3)),)}
            f31 = {0: (C(out_back, 0),), 2: (C(out_back, 1),),
                   4: (C(out_back, 2),), 6: (C(out_back, 3),)}

            pair_order = [(0, 0), (1, 0), (2, 0), (3, 0),
                          (0, 1), (1, 1), (2, 1), (3, 1)]
            fillmap = {(0, 0): f00, (1, 0): f10, (2, 0): f20,
                       (0, 1): f01, (1, 1): f11, (2, 1): f21, (3, 1): f31}
            attend_all(pair_order, fillmap)
            out_front(4)
            out_front(5)
            out_front(6)
            out_front(7)
            rstd_batch((4, 5, 6, 7))
            out_back(4, tail=True)
            out_back(5, tail=True)
            out_back(6, tail=True)
            out_back(7, tail=True)

    nc.compile()
    return nc


def kernel(**inputs):
    from concourse.bass_utils import run_bass_kernel_spmd

    gamma_a = np.asarray(inputs["gamma"], dtype=np.float32)
    beta_a = np.asarray(inputs["beta"], dtype=np.float32)
    ln_affine = bool(np.any(gamma_a != 1.0) or np.any(beta_a != 0.0))
    ck = ("nc", ln_affine)
    if ck not in _CACHE:
        _CACHE[ck] = _build(ln_affine)
    nc = _CACHE[ck]

    query = np.asarray(inputs["query"], dtype=np.float32)
    key = np.asarray(inputs["key"], dtype=np.float32)
    value = np.asarray(inputs["value"], dtype=np.float32)
    mask = np.asarray(inputs["mask"])
    WQ = np.asarray(inputs["WQ"], dtype=np.float32)
    WK = np.asarray(inputs["WK"], dtype=np.float32)
    WV = np.asarray(inputs["WV"], dtype=np.float32)
    WO = np.asarray(inputs["WO"], dtype=np.float32)
    bO = np.asarray(inputs["bO"], dtype=np.float32)
    gamma = np.asarray(inputs["gamma"], dtype=np.float32)
    beta = np.asarray(inputs["beta"], dtype=np.float32)

    scale = np.float32(1.0 / np.sqrt(HD))
    wqT = np.ascontiguousarray(WQ.T * scale).astype(BF16)
    wkT = np.ascontiguousarray(WK.T).astype(BF16)
    wvT = np.ascontiguousarray(WV.T).astype(BF16)
    woT = np.ascontiguousarray(WO.T).astype(BF16)
    gamma_in = gamma.reshape(1, D)
    beta_in = beta.reshape(1, D)
    mask_bin = (mask != 0)

    in_maps = []
    for c in range(NCORES):
        b, n0 = c // 2, (c % 2) * NS
        # mask, transposed and prepacked per (n-chunk, score-group):
        # maskP[ncc, g, p, u*512+nn] = maskT[g*256+u*128+p, ncc*512+nn]
        mT = np.ascontiguousarray(mask_bin[b, n0 : n0 + NS, :].T)  # [M, NS]
        mP = (
            mT.reshape(8, 2, 128, 2, 512)
            .transpose(3, 0, 2, 1, 4)
            .reshape(2 * 8 * 128, 1024)
        )
        in_maps.append({
            "xqT": np.ascontiguousarray(query[b, n0 : n0 + NS, :].T).astype(BF16),
            "xkT": np.ascontiguousarray(key[b].T).astype(BF16),
            "xvT": np.ascontiguousarray(value[b].T).astype(BF16),
            "maskP": np.ascontiguousarray(mP).astype(BF16),
            "qres": np.ascontiguousarray(query[b, n0 : n0 + NS, :] + bO[None, :]),
            "wqT": wqT, "wkT": wkT, "wvT": wvT, "woT": woT,
            "gamma": gamma_in, "beta": beta_in,
        })

    trace = bool(int(os.environ.get("BASS_KERNEL_TRACE", "0")))
    res = run_bass_kernel_spmd(nc, in_maps, core_ids=list(range(NCORES)), trace=trace)
    _CACHE["last_results"] = res

    out = np.empty((B, N, D), dtype=np.float32)
    for c in range(NCORES):
        b, n0 = c // 2, (c % 2) * NS
        out[b, n0 : n0 + NS, :] = res.results[c]["out"]
    return out
